# revision 1
# baseline (speedup 1.0000x reference)
"""DGAT attention head on 8 trn2 NeuronCores.

Sharding: row-wise over query nodes (core c owns rows [c*R, (c+1)*R)).
Each core receives its adj slice pre-transposed and mask-encoded
(host-side layout choice): adjt_enc = (adj^T - 1) * BIG in bf16
(exact: adj is binary), so masking becomes an additive logit bias.

Math (exact for binary adj):
  h   = x @ w;  hz1 = x @ (w @ a[:D]);  hz2 = x @ (w @ a[D:])
  z   = C*(hz1[i] + hz2[j]) + D0;  L1 = leaky(A+B)   (leaky slope 0.2)
  row-softmax of masked logits  ==  normalize(exp(L1*leaky(z) - G + BIG*(adj-1)))
  out = elu((p @ h) / (p @ 1))
G is a host-derived bound on max logit (from hz1/hz2 extremes).

Device pipeline per 256-j megatile (j on partitions, i free):
  DMA 512KB bf16 adjt_enc
  -> custom DVE op: u = leakyscaled(Src0 + bias_col) + adjt_enc - G   (1 pass)
  -> ACT Exp: q = exp(u), bf16 out                                    (1 pass)
  -> PE: psum[65, R] += [h|1]^T-group @ q-half (bf16, N=1024)
Tail: psum -> sbuf, PE transposes [65,128]->[128,65], reciprocal of the
sum column, ELU = relu(v) + (min(exp(v),1) - 1).

The leaky-scale trick: for L1>=0, u_leak = select(w>=0, w, 0.2w) with
w = L1*z (positive homogeneity); for L1<0, w = 0.2*L1*z and the false
branch multiplies by 1/0.2.
"""

import numpy as np
import ml_dtypes

import concourse.bass as bass
import concourse.bacc as bacc
import concourse.mybir as mybir
import concourse.dve_ops as dve_ops
from concourse.dve_spec import Spec, Src0, Src1, C0, C1, C2, Zero, One, select, maxx
from concourse.tile import TileContext
from concourse.bass_utils import run_bass_kernel_spmd

F32 = mybir.dt.float32
F16 = mybir.dt.float16
F32R = mybir.dt.float32r
F8 = mybir.dt.float8e5
AF = mybir.ActivationFunctionType
OP = mybir.AluOpType

NCORES = 8
SLOPE = 0.2    # leakyrelu negative slope (fixed in the reference)
BIG = 16384.0  # additive mask magnitude (exact in fp16; exp(-BIG) == 0)

TRACE = False
LAST_RESULTS = None
LAST_NC = None


def _leaky(z):
    return z if z >= 0.0 else SLOPE * z


def _register_leaky_mask_op():
    name = "LEAKY_MASK_BIAS_ANT"
    for op in dve_ops.OPS:
        if op.name == name:
            return op
    w = Src0 + C0
    spec = Spec(
        body=select(w >= Zero, w, w * C1) + Src1 + C2,
        reference=lambda in0, in1, s0, s1, imm2: (
            np.where(in0 + s0 >= 0, in0 + s0, (in0 + s0) * s1) + in1 + imm2
        ).astype(np.float32),
    )
    return _finish_register(name, spec)


def _finish_register(name, spec):
    from concourse.dve_spec import lower
    from concourse.dve_ops import has_src1
    from concourse.dve_uop import DveOpSpec

    op = dve_ops.DveOp(name, spec, subdim=False, uops_sha={})
    dve_ops.OPS.append(op)
    dve_ops.CUSTOM_DVE_SPECS[name] = spec
    dve_ops._SUB_OPCODE_FOR_NAME[name] = (
        dve_ops._CUSTOM_DVE_ROW_BASE + len(dve_ops.OPS) - 1
    )
    assert dve_ops._SUB_OPCODE_FOR_NAME[name] < 0x20
    for ver in ("v3",):
        pinned = DveOpSpec(
            name=name,
            opcode=dve_ops.get_dve_sub_opcode(name),
            uops=lower(spec, ver=ver),
            rd1_en=has_src1(spec),
        ).sha(ver)
        op.uops_sha[ver] = pinned
        dve_ops._COMPILE_CACHE.pop((name, ver), None)
        op.compile(ver)
    return op


def _register_elu_max_op():
    name = "ELU_MAX_ANT"
    for op in dve_ops.OPS:
        if op.name == name:
            return op
    spec = Spec(
        body=maxx(Src0 * C0, Src1 - One),
        reference=lambda in0, in1, s0, s1, imm2: np.maximum(
            in0 * s0, in1 - 1.0
        ).astype(np.float32),
    )
    return _finish_register(name, spec)


def _build(n, din, dout, rows, kpre, s1_slope, G):
    """Build the SPMD Bass program (identical on all cores).

    kpre: scale applied to hz1/hz2 logit halves (= k*C with k = L1 or
    SLOPE*L1); the per-partition bias col is kpre*hz2 + kD (kD folded on
    device); s1_slope: false-branch slope of the select (0.2 or 5.0).
    """
    assert n % 256 == 0 and rows % 128 == 0 and din % 128 == 0
    ng = n // 128
    mt = 4 if n % 512 == 0 else 2
    nm = n // (128 * mt)
    kc = din // 128
    grp = 4
    assert ng % grp == 0
    de = dout + 1
    lmb = _register_leaky_mask_op()
    emx = _register_elu_max_op()

    nc = bacc.Bacc("TRN2", target_bir_lowering=False)
    adjt_d = nc.dram_tensor("adjt", [n, rows], F8, kind="ExternalInput")
    xt_d = nc.dram_tensor("xt", [din, n], F16, kind="ExternalInput")
    xto_d = nc.dram_tensor("xt_own", [din, rows], F16, kind="ExternalInput")
    w_d = nc.dram_tensor("w", [din, dout], F32, kind="ExternalInput")
    a_d = nc.dram_tensor("a", [2 * dout, 1], F32, kind="ExternalInput")
    kd_d = nc.dram_tensor("kd", [1, 1], F32, kind="ExternalInput")
    y_d = nc.dram_tensor("y", [rows, dout], F32, kind="ExternalOutput")

    with TileContext(nc) as tc:
        with (
            tc.tile_pool(name="consts", bufs=1) as consts,
            tc.tile_pool(name="adjp", bufs=8) as adjp,
            tc.tile_pool(name="up", bufs=2) as up,
            tc.tile_pool(name="qp", bufs=2) as qp,
            tc.tile_pool(name="et2p", bufs=1) as et2p,
            tc.tile_pool(name="tailp", bufs=2) as tailp,
        ):
            from concourse.masks import make_identity

            identity0 = consts.tile([128, 128], F32)
            make_identity(nc, identity0)
            identity = consts.tile([128, 128], F32)
            nc.vector.tensor_copy(identity, identity0)

            zcol = consts.tile([128, 1], F32)
            nc.vector.memset(zcol, 0.0)
            negGcol = consts.tile([128, 1], F32)
            nc.vector.memset(negGcol, -G)
            ones128 = consts.tile([128, 128], F16)
            nc.vector.memset(ones128, 1.0)
            # kD broadcast column (k*D0 replicated to all partitions)
            kdcol = consts.tile([128, 1], F32)
            kd_ap = kd_d[:, :]
            nc.sync.dma_start(
                out=kdcol,
                in_=bass.AP(tensor=kd_ap.tensor, offset=0, ap=[[0, 128], [1, 1]]),
            )

            # a1/a2 broadcast across partitions (partition-step-0 DMA)
            a_ap = a_d[:, :]
            a1bc = consts.tile([128, dout], F32)
            nc.sync.dma_start(
                out=a1bc,
                in_=bass.AP(tensor=a_ap.tensor, offset=0, ap=[[0, 128], [1, dout]]),
            )
            a2bc = consts.tile([128, dout], F32)
            nc.sync.dma_start(
                out=a2bc,
                in_=bass.AP(
                    tensor=a_ap.tensor, offset=dout, ap=[[0, 128], [1, dout]]
                ),
            )

            # wx_k = [w_k | w_k@a1 | w_k@a2] in bf16, single DVE writer
            wx = []
            wxraw = []
            for k in range(kc):
                wxr = consts.tile([128, dout + 2], F32, name=f"wxr{k}")
                nc.sync.dma_start(
                    out=wxr[:, 0:dout], in_=w_d[k * 128 : (k + 1) * 128, :]
                )
                t1 = consts.tile([128, dout], F32, name=f"wa_t{k}")
                nc.vector.tensor_mul(t1, wxr[:, 0:dout], a1bc)
                nc.vector.reduce_sum(
                    wxr[:, dout : dout + 1], t1, axis=mybir.AxisListType.X
                )
                t2 = consts.tile([128, dout], F32, name=f"wb_t{k}")
                nc.vector.tensor_mul(t2, wxr[:, 0:dout], a2bc)
                nc.vector.reduce_sum(
                    wxr[:, dout + 1 : dout + 2], t2, axis=mybir.AxisListType.X
                )
                wxk = consts.tile([128, dout + 2], F16, name=f"wx{k}")
                nc.vector.tensor_copy(wxk, wxr)
                wx.append(wxk)
                wxraw.append(wxr)

            h_ext = consts.tile([128, ng, de], F32R)
            # memset can't write f32r; copy from a ones tile instead
            nc.vector.tensor_copy(
                h_ext[:, :, dout : dout + 1], ones128[:, 0:ng]
            )
            hz1bc = consts.tile([128, rows], F32)
            hz2cols = consts.tile([128, ng], F32)
            bias_cols = consts.tile([128, ng], F32)
            hpT = consts.tile([de, rows], F32)

            with (
                tc.tile_pool(name="xtp", bufs=1) as xtp,
                tc.tile_pool(name="pshz", bufs=1, space="PSUM") as pshz,
                tc.tile_pool(name="pspre", bufs=2, space="PSUM") as pspre,
            ):
                # own-x columns + hz1 broadcast first: this unblocks the
                # main-loop custom ops as early as possible
                xtos = []
                for k in range(kc):
                    xtok = xtp.tile([128, rows], F16, name=f"xto{k}")
                    nc.sync.dma_start(
                        out=xtok, in_=xto_d[k * 128 : (k + 1) * 128, :]
                    )
                    xtos.append(xtok)
                hz_ps = pshz.tile([128, rows], F32)
                for k in range(kc):
                    wa1bc = consts.tile([128, 128], F16, name=f"wa1bc{k}")
                    nc.vector.tensor_scalar_mul(
                        wa1bc, ones128, wxraw[k][:, dout : dout + 1]
                    )
                    nwmax = 512
                    for n0 in range(0, rows, nwmax):
                        nw = min(nwmax, rows - n0)
                        nc.tensor.matmul(
                            hz_ps[:, n0 : n0 + nw],
                            wa1bc,
                            xtos[k][:, n0 : n0 + nw],
                            start=(k == 0),
                            stop=(k == kc - 1),
                        )
                nc.vector.tensor_scalar_mul(hz1bc, hz_ps, kpre)

                xchunk = 2048 if n >= 2048 else n
                xts = [
                    xtp.tile([128, n], F16, name=f"xt{k}") for k in range(kc)
                ]
                for c0 in range(0, n, xchunk):
                    for k in range(kc):
                        nc.sync.dma_start(
                            out=xts[k][:, c0 : c0 + xchunk],
                            in_=xt_d[k * 128 : (k + 1) * 128, c0 : c0 + xchunk],
                        )

                # h_ext (f32r), hz2 and bias columns per j-group, in the
                # order the main loop consumes them
                for g0 in range(0, ng, grp):
                    ps = pspre.tile([128, grp, dout + 2], F32, name="ps_h")
                    for gi in range(grp):
                        g = g0 + gi
                        for k in range(kc):
                            nc.tensor.matmul(
                                ps[:, gi, :],
                                xts[k][:, g * 128 : (g + 1) * 128],
                                wx[k],
                                start=(k == 0),
                                stop=(k == kc - 1),
                            )
                    nc.scalar.copy(
                        h_ext[:, g0 : g0 + grp, 0:dout], ps[:, :, 0:dout]
                    )
                    nc.scalar.copy(
                        hz2cols[:, g0 : g0 + grp],
                        ps[:, :, dout + 1 : dout + 2],
                    )
                    nc.vector.tensor_scalar(
                        bias_cols[:, g0 : g0 + grp],
                        hz2cols[:, g0 : g0 + grp],
                        kpre,
                        kdcol[:, 0:1],
                        OP.mult,
                        OP.add,
                    )

            # main loop: stream encoded adjT megatiles (256 j x rows i)
            adjt_r = adjt_d[:, :].rearrange(
                "(m t p) i -> m p t i", t=mt, p=128
            )
            with (
                tc.tile_pool(name="psacc", bufs=1, space="PSUM") as psacc,
                tc.tile_pool(name="pstail", bufs=4, space="PSUM") as pstail,
            ):
                acc = psacc.tile([de, rows], F32)
                # last megatile runs leaky on ACT (Prelu) + mask-add on
                # GPSIMD to relieve the DVE bottleneck
                gp_ms = set()  # GP-assist measured slower in timeline sim
                for m in range(nm):
                    adjt_t = adjp.tile([128, mt * rows], F8)
                    nc.sync.dma_start(
                        out=adjt_t.rearrange("p (t i) -> p t i", t=mt),
                        in_=adjt_r[m],
                    )
                    u = up.tile([128, mt * rows], F32)
                    if m in gp_ms:
                        et2 = et2p.tile([128, mt * rows], F32)
                        for t in range(mt):
                            g = mt * m + t
                            nc.scalar.activation(
                                et2[:, t * rows : (t + 1) * rows],
                                hz1bc,
                                AF.Prelu,
                                bias=bias_cols[:, g : g + 1],
                                alpha=s1_slope,
                            )
                        for t in range(mt):
                            sl = slice(t * rows, (t + 1) * rows)
                            nc.gpsimd.tensor_add(
                                u[:, sl], et2[:, sl], adjt_t[:, sl]
                            )
                        expbias = negGcol
                    else:
                        for t in range(mt):
                            g = mt * m + t
                            nc.vector._custom_dve(
                                lmb,
                                out=u[:, t * rows : (t + 1) * rows],
                                in0=hz1bc,
                                in1=adjt_t[:, t * rows : (t + 1) * rows],
                                s0=bias_cols[:, g : g + 1],
                                s1=s1_slope,
                                imm2=-G,
                            )
                        expbias = zcol
                    q = qp.tile([128, mt * rows], F32R)
                    for t0 in range(0, mt, 2):
                        nc.scalar.activation(
                            q[:, t0 * rows : (t0 + 2) * rows],
                            u[:, t0 * rows : (t0 + 2) * rows],
                            AF.Exp,
                            bias=expbias[:, 0:1],
                        )
                    for t in range(mt):
                        g = mt * m + t
                        nwmax = 512
                        for n0 in range(0, rows, nwmax):
                            nw = min(nwmax, rows - n0)
                            nc.tensor.matmul(
                                acc[:, n0 : n0 + nw],
                                h_ext[:, g, :],
                                q[:, t * rows + n0 : t * rows + n0 + nw],
                                start=(g == 0),
                                stop=(g == ng - 1),
                            )

                # tail: normalize + elu, back to i-major.
                # hpT row de holds 1/s so each transposed chunk carries its
                # per-partition reciprocal in column de.
                nc.scalar.copy(hpT[0:dout, :], acc[0:dout, :])
                nc.vector.reciprocal(hpT[dout:de, :], acc[dout:de, :])
                for cc in range(rows // 128):
                    tp = pstail.tile([128, de], F32)
                    nc.tensor.transpose(
                        tp,
                        hpT[:, cc * 128 : (cc + 1) * 128],
                        identity[0:de, 0:de],
                    )
                    # elu(v) = max(v, exp(min(v, 0)) - 1), v = hp * (1/s)
                    vm = tailp.tile([128, dout], F32)
                    nc.vector.tensor_scalar(
                        vm, tp[:, 0:dout], tp[:, dout:de], 0.0,
                        OP.mult, OP.min,
                    )
                    e2 = tailp.tile([128, dout], F32)
                    nc.scalar.activation(e2, vm, AF.Exp, bias=zcol[:, 0:1])
                    ysb = tailp.tile([128, dout], F32)
                    nc.vector._custom_dve(
                        emx, out=ysb, in0=tp[:, 0:dout], in1=e2,
                        s0=tp[:, dout:de], s1=0.0, imm2=0.0,
                    )
                    nc.sync.dma_start(
                        out=y_d[cc * 128 : (cc + 1) * 128, :], in_=ysb
                    )
    nc.compile()
    return nc


def _run(x, adj, w, a, a_coeff, b_coeff, c_coeff, d_coeff):
    global LAST_RESULTS, LAST_NC
    n, din = x.shape
    dout = w.shape[1]
    assert adj.shape == (n, n) and a.shape == (2 * dout, 1)
    rows = n // NCORES

    A = float(np.asarray(a_coeff).reshape(-1)[0])
    B = float(np.asarray(b_coeff).reshape(-1)[0])
    C = float(np.asarray(c_coeff).reshape(-1)[0])
    D0 = float(np.asarray(d_coeff).reshape(-1)[0])
    L1 = _leaky(A + B)

    x = np.ascontiguousarray(x, dtype=np.float32)
    adj = np.asarray(adj, dtype=np.float32)
    # the mask-encoding algebra requires a binary adjacency
    assert ((adj == 0.0) | (adj == 1.0)).all(), "adj must be binary"
    w = np.ascontiguousarray(w, dtype=np.float32)
    a = np.ascontiguousarray(a, dtype=np.float32)

    # host-side stability shift G >= max logit (from h extremes only)
    h = x @ w
    hz1 = h @ a[:dout, 0]
    hz2 = h @ a[dout:, 0]
    cand = []
    for u in (hz1.min(), hz1.max()):
        for v in (hz2.min(), hz2.max()):
            cand.append(L1 * _leaky(C * (float(u) + float(v)) + D0))
    G = float(max(cand))

    # leaky-scale trick (positive homogeneity of leaky)
    if L1 >= 0.0:
        kk, s1_slope = L1, SLOPE
    else:
        kk, s1_slope = SLOPE * L1, 1.0 / SLOPE
    kpre = kk * C

    nc = _build(n, din, dout, rows, kpre, s1_slope, G)
    LAST_NC = nc

    xt_b = np.ascontiguousarray(x.T).astype(np.float16)
    kd = np.full((1, 1), kk * D0, dtype=np.float32)
    in_maps = []
    for c in range(NCORES):
        sl = slice(c * rows, (c + 1) * rows)
        adjt_enc = ((adj[sl, :].T - 1.0) * BIG).astype(ml_dtypes.float8_e5m2)
        in_maps.append(
            {
                "adjt": np.ascontiguousarray(adjt_enc),
                "xt": xt_b,
                "xt_own": np.ascontiguousarray(xt_b[:, sl]),
                "w": w,
                "a": a,
                "kd": kd,
            }
        )

    res = run_bass_kernel_spmd(
        nc, in_maps, core_ids=list(range(NCORES)), trace=TRACE
    )
    LAST_RESULTS = res
    return np.concatenate([r["y"] for r in res.results], axis=0)


def kernel(x, adj, w, a, a_coeff, b_coeff, c_coeff, d_coeff):
    return _run(x, adj, w, a, a_coeff, b_coeff, c_coeff, d_coeff)



# revision 18
# speedup vs baseline: 1.6603x; 1.6603x over previous
"""DGAT attention head on 8 trn2 NeuronCores — separable-logit version.

Math: logit_ij = L1*leaky(C*(hz1_i + hz2_j) + D0) for adj_ji=1 (masked else),
with L1 = leaky(A+B) constant. Since leaky(y) = max(y, SLOPE*y) and exp is
monotone, the unnormalized softmax weight factorizes per branch:
  q_ji = adj_ji * sel(F1_j, R_i * F2_j)        sel = max if L1>=0 else min
  F1_j = exp(k1*hz2_j + d1 - s0), F2_j = exp(k2*hz2_j + d2 - s0),
  R_i  = exp((k2-k1)*hz1_i), k1 = L1*C, k2 = L1*C*SLOPE
(the per-query factor exp(k1*hz1_i + s0) cancels in the softmax ratio).

Host sorts j by hz2 (globally) and assigns core c the i's with hz1-rank in
[c*R, (c+1)*R) (sorted). Then per j-group g of 128 sorted j's the branch
boundary over sorted i-columns is monotone: columns [0, z2_g) are pure
branch-2, [z2_g, z1_g) mixed, [z1_g, 1024) pure branch-1. Pure ranges are
plain 0/1 fp8 adjacency matmuls against F-scaled bf16 stationaries:
  acc1 += adjT_g @ (F1 ⊙ [h|1])   (branch-1 cols)
  acc2 += adjT_g @ (F2 ⊙ [h|1])   (branch-2 cols; R_i applied at combine)
Mixed cols use moving q_b = adj ⊙ sel(1, R_i*rho_j), rho = F2/F1, against the
F1 stationary. Combine: hp = acc1 + R ⊙ acc2, then normalize + ELU as before.
Elementwise full-matrix work is gone; kernel is DMA/PE bound.
"""

import numpy as np

import concourse.bass as bass
import concourse.bacc as bacc
import concourse.mybir as mybir
import concourse.dve_ops as dve_ops
from concourse.dve_spec import Spec, Src0, Src1, maxx
from concourse.tile import TileContext
from concourse.bass_utils import run_bass_kernel_spmd

F32 = mybir.dt.float32
F16 = mybir.dt.float16
BF16 = mybir.dt.bfloat16
F8E4 = mybir.dt.float8e4
AF = mybir.ActivationFunctionType
OP = mybir.AluOpType

NCORES = 8
SLOPE = 0.2  # leakyrelu negative slope (fixed in the reference)

TRACE = False
LAST_RESULTS = None
LAST_NC = None


def _leaky(z):
    return z if z >= 0.0 else SLOPE * z


def _register_elu_max_op():
    name = "ELU_MAX_ANT"
    for op in dve_ops.OPS:
        if op.name == name:
            return op
    from concourse.dve_spec import C0, One

    spec = Spec(
        body=maxx(Src0 * C0, Src1 - One),
        reference=lambda in0, in1, s0, s1, imm2: np.maximum(
            in0 * s0, in1 - 1.0
        ).astype(np.float32),
    )
    return _finish_register(name, spec)


def _finish_register(name, spec):
    from concourse.dve_spec import lower
    from concourse.dve_ops import has_src1
    from concourse.dve_uop import DveOpSpec

    op = dve_ops.DveOp(name, spec, subdim=False, uops_sha={})
    dve_ops.OPS.append(op)
    dve_ops.CUSTOM_DVE_SPECS[name] = spec
    dve_ops._SUB_OPCODE_FOR_NAME[name] = (
        dve_ops._CUSTOM_DVE_ROW_BASE + len(dve_ops.OPS) - 1
    )
    assert dve_ops._SUB_OPCODE_FOR_NAME[name] < 0x20
    for ver in ("v3",):
        pinned = DveOpSpec(
            name=name,
            opcode=dve_ops.get_dve_sub_opcode(name),
            uops=lower(spec, ver=ver),
            rd1_en=has_src1(spec),
        ).sha(ver)
        op.uops_sha[ver] = pinned
        dve_ops._COMPILE_CACHE.pop((name, ver), None)
        op.compile(ver)
    return op


def _build(n, din, dout, rows, consts_d):
    """Build the SPMD Bass program (identical on all cores).

    consts_d: dict with floats k1, k2, b_f1 (=d1-s0), b_rho (=d2-d1),
    sel_max (bool), z2/z1 (per-group column split ints).
    """
    assert n % 512 == 0 and rows % 128 == 0 and din % 128 == 0
    ng = n // 128
    mt = 4
    nm = ng // mt
    kc = din // 128
    de = dout + 1      # [h | F] stationary width
    emx = _register_elu_max_op()

    k1 = consts_d["k1"]
    k2 = consts_d["k2"]
    b_f1 = consts_d["b_f1"]
    b_rho = consts_d["b_rho"]
    selop = OP.max if consts_d["sel_max"] else OP.min
    Z2 = consts_d["z2"]
    Z1 = consts_d["z1"]

    # last group index that emits a matmul into each accumulator (for stop=)
    last_g1 = max(
        (g for g in range(ng) if Z1[g] < rows or Z2[g] < Z1[g]), default=None
    )
    last_g2 = max((g for g in range(ng) if Z2[g] > 0), default=None)

    nc = bacc.Bacc("TRN2", target_bir_lowering=False)
    adjt_d = nc.dram_tensor("adjt", [n, rows], F8E4, kind="ExternalInput")
    xt_d = nc.dram_tensor("xt", [din, n], F16, kind="ExternalInput")
    xto_d = nc.dram_tensor("xt_own", [din, rows], F16, kind="ExternalInput")
    w_d = nc.dram_tensor("w", [din, dout], F32, kind="ExternalInput")
    a_d = nc.dram_tensor("a", [2 * dout, 1], F32, kind="ExternalInput")
    y_d = nc.dram_tensor("y", [rows, dout], F32, kind="ExternalOutput")

    with TileContext(nc) as tc:
        with (
            tc.tile_pool(name="consts", bufs=1) as consts,
            tc.tile_pool(name="adjp", bufs=6) as adjp,
            tc.tile_pool(name="bp", bufs=3) as bp,
            tc.tile_pool(name="tailp", bufs=2) as tailp,
        ):
            from concourse.masks import make_identity

            identity = consts.tile([128, 128], F32)
            make_identity(nc, identity)

            zcol = consts.tile([128, 1], F32)
            nc.vector.memset(zcol, 0.0)
            bf1col = consts.tile([128, 1], F32)
            nc.vector.memset(bf1col, b_f1)
            brhocol = consts.tile([128, 1], F32)
            nc.vector.memset(brhocol, b_rho)
            ones128 = consts.tile([128, 128], F16)
            nc.vector.memset(ones128, 1.0)
            zstat = consts.tile([128, de], BF16)
            nc.vector.memset(zstat, 0.0)
            zmov = consts.tile([128, rows], BF16)
            nc.gpsimd.memset(zmov, 0.0)

            # a1/a2 broadcast across partitions (partition-step-0 DMA)
            a_ap = a_d[:, :]
            a1bc = consts.tile([128, dout], F32)
            nc.sync.dma_start(
                out=a1bc,
                in_=bass.AP(tensor=a_ap.tensor, offset=0, ap=[[0, 128], [1, dout]]),
            )
            a2bc = consts.tile([128, dout], F32)
            nc.sync.dma_start(
                out=a2bc,
                in_=bass.AP(
                    tensor=a_ap.tensor, offset=dout, ap=[[0, 128], [1, dout]]
                ),
            )

            # wx_k = [w_k | w_k@a2] f16 stationaries for the h/hz2 compute;
            # wa1 column feeds the hz1 broadcast trick.
            wx = []
            wa1cols = []
            for k in range(kc):
                wxr = consts.tile([128, dout + 2], F32, name=f"wxr{k}")
                nc.sync.dma_start(
                    out=wxr[:, 0:dout], in_=w_d[k * 128 : (k + 1) * 128, :]
                )
                t1 = consts.tile([128, dout], F32, name=f"wa_t{k}")
                nc.vector.tensor_mul(t1, wxr[:, 0:dout], a1bc)
                nc.vector.reduce_sum(
                    wxr[:, dout : dout + 1], t1, axis=mybir.AxisListType.X
                )
                t2 = consts.tile([128, dout], F32, name=f"wb_t{k}")
                nc.vector.tensor_mul(t2, wxr[:, 0:dout], a2bc)
                nc.vector.reduce_sum(
                    wxr[:, dout + 1 : dout + 2], t2, axis=mybir.AxisListType.X
                )
                wxk = consts.tile([128, dout + 1], F16, name=f"wx{k}")
                nc.vector.tensor_copy(wxk[:, 0:dout], wxr[:, 0:dout])
                nc.vector.tensor_copy(
                    wxk[:, dout : dout + 1], wxr[:, dout + 1 : dout + 2]
                )
                wx.append(wxk)
                wa1cols.append(wxr)

            # stationaries and per-group columns
            h1s = consts.tile([128, ng, de], BF16)
            h2s = consts.tile([128, ng, de], BF16)
            f1cols = consts.tile([128, ng], F32)
            rhocols = consts.tile([128, ng], F32)
            Rbc = consts.tile([128, rows], BF16)
            hpT = consts.tile([de, rows], F32)

            with (
                tc.tile_pool(name="xtp", bufs=1) as xtp,
                tc.tile_pool(name="pshz", bufs=1, space="PSUM") as pshz,
                tc.tile_pool(name="pspre", bufs=2, space="PSUM") as pspre,
                tc.tile_pool(name="psacc", bufs=1, space="PSUM") as psacc,
            ):
                # --- own-x columns -> hz1 broadcast -> R broadcast ---
                xtos = []
                for k in range(kc):
                    xtok = xtp.tile([128, rows], F16, name=f"xto{k}")
                    nc.sync.dma_start(
                        out=xtok, in_=xto_d[k * 128 : (k + 1) * 128, :]
                    )
                    xtos.append(xtok)
                hz_ps = pshz.tile([128, rows], F32)
                for k in range(kc):
                    wa1bc = consts.tile([128, 128], F16, name=f"wa1bc{k}")
                    nc.vector.tensor_scalar_mul(
                        wa1bc, ones128, wa1cols[k][:, dout : dout + 1]
                    )
                    for n0 in range(0, rows, 512):
                        nc.tensor.matmul(
                            hz_ps[:, n0 : n0 + 512],
                            wa1bc,
                            xtos[k][:, n0 : n0 + 512],
                            start=(k == 0),
                            stop=(k == kc - 1),
                        )
                # R_i = exp((k2-k1)*hz1_i), broadcast on all partitions
                nc.scalar.activation(
                    Rbc, hz_ps, AF.Exp, bias=zcol[:, 0:1], scale=k2 - k1
                )

                # --- full x (j-sorted) -> per-group stationaries ---
                xchunk = 2048
                xts = [
                    xtp.tile([128, n], F16, name=f"xt{k}") for k in range(kc)
                ]
                for c0 in range(0, n, xchunk):
                    for k in range(kc):
                        nc.sync.dma_start(
                            out=xts[k][:, c0 : c0 + xchunk],
                            in_=xt_d[k * 128 : (k + 1) * 128, c0 : c0 + xchunk],
                        )

                grp = 4
                for g0 in range(0, ng, grp):
                    ps = pspre.tile([128, grp, de], F32, name="ps_h")
                    for gi in range(grp):
                        g = g0 + gi
                        for k in range(kc):
                            nc.tensor.matmul(
                                ps[:, gi, :],
                                xts[k][:, g * 128 : (g + 1) * 128],
                                wx[k],
                                start=(k == 0),
                                stop=(k == kc - 1),
                            )
                    # F1 into h1s[:, g, dout] (strided), rho into rhocols
                    nc.scalar.activation(
                        f1cols[:, g0 : g0 + grp],
                        ps[:, :, dout : dout + 1],
                        AF.Exp,
                        bias=bf1col[:, 0:1],
                        scale=k1,
                    )
                    nc.scalar.activation(
                        h1s[:, g0 : g0 + grp, dout : dout + 1],
                        ps[:, :, dout : dout + 1],
                        AF.Exp,
                        bias=bf1col[:, 0:1],
                        scale=k1,
                    )
                    nc.scalar.activation(
                        rhocols[:, g0 : g0 + grp],
                        ps[:, :, dout : dout + 1],
                        AF.Exp,
                        bias=brhocol[:, 0:1],
                        scale=k2 - k1,
                    )
                    for gi in range(grp):
                        g = g0 + gi
                        # h1s[:, g, :dout] = F1_j * h ; col dout already = F1_j
                        nc.scalar.activation(
                            h1s[:, g, 0:dout],
                            ps[:, gi, 0:dout],
                            AF.Copy,
                            bias=0.0,
                            scale=f1cols[:, g : g + 1],
                        )
                        # h2s = rho_j * h1s (including the F column)
                        nc.vector.tensor_scalar_mul(
                            h2s[:, g, :], h1s[:, g, :], rhocols[:, g : g + 1]
                        )

                # --- main loop: stream adjT megatiles, accumulate ---
                acc1 = psacc.tile([de, rows], F32)
                acc2 = psacc.tile([de, rows], F32)
                def acc_mm(acc, stat, mv, c0, c1, start, stop):
                    # matmul outputs may not cross the 512-col PSUM bank edge
                    for b0 in range(0, rows, 512):
                        lo = max(c0, b0)
                        hi = min(c1, b0 + 512)
                        if lo < hi:
                            nc.tensor.matmul(
                                acc[:, lo:hi],
                                stat,
                                mv[:, lo:hi],
                                start=start,
                                stop=stop,
                                skip_group_check=True,
                            )

                acc_mm(acc1, zstat, zmov, 0, rows, True, last_g1 is None)
                acc_mm(acc2, zstat, zmov, 0, rows, True, last_g2 is None)
                adjt_r = adjt_d[:, :].rearrange(
                    "(m t p) i -> m p t i", t=mt, p=128
                )
                for m in range(nm):
                    adjt_t = adjp.tile([128, mt, rows], F8E4)
                    nc.sync.dma_start(out=adjt_t, in_=adjt_r[m])
                    for t in range(mt):
                        g = mt * m + t
                        z2, z1 = Z2[g], Z1[g]
                        mv = adjt_t[:, t, :]
                        if z2 > 0:
                            acc_mm(
                                acc2, h2s[:, g, :], mv, 0, z2,
                                False, g == last_g2,
                            )
                        if z1 > z2:
                            # mixed columns: q_b = adj * sel(1, R_i*rho_j)
                            tb = bp.tile([128, z1 - z2], BF16, name="tb")
                            nc.vector.tensor_scalar(
                                tb,
                                Rbc[:, z2:z1],
                                rhocols[:, g : g + 1],
                                1.0,
                                OP.mult,
                                selop,
                            )
                            qb = bp.tile([128, z1 - z2], BF16, name="qb")
                            nc.vector.tensor_mul(qb, tb, mv[:, z2:z1])
                            for b0 in range(0, rows, 512):
                                lo = max(z2, b0)
                                hi = min(z1, b0 + 512)
                                if lo < hi:
                                    nc.tensor.matmul(
                                        acc1[:, lo:hi],
                                        h1s[:, g, :],
                                        qb[:, lo - z2 : hi - z2],
                                        start=False,
                                        stop=(
                                            g == last_g1 and z1 == rows
                                            and hi == z1
                                        ),
                                        skip_group_check=True,
                                    )
                        if z1 < rows:
                            acc_mm(
                                acc1, h1s[:, g, :], mv, z1, rows,
                                False, g == last_g1,
                            )

                # --- combine: hp = acc1 + R ⊙ acc2 (row dout = s) ---
                t2 = consts.tile([de, rows], F32, name="combine_t2")
                nc.vector.tensor_mul(t2, acc2, Rbc[0:de, :])
                nc.vector.tensor_add(hpT, t2, acc1)

            with tc.tile_pool(name="pstail", bufs=4, space="PSUM") as pstail:
                for cc in range(rows // 128):
                    tp = pstail.tile([128, de], F32)
                    nc.tensor.transpose(
                        tp,
                        hpT[:, cc * 128 : (cc + 1) * 128],
                        identity[0:de, 0:de],
                    )
                    rcol = tailp.tile([128, 1], F32)
                    nc.vector.reciprocal(rcol, tp[:, dout:de])
                    # elu(v) = max(v, exp(min(v, 0)) - 1), v = hp * (1/s)
                    vm = tailp.tile([128, dout], F32)
                    nc.vector.tensor_scalar(
                        vm, tp[:, 0:dout], rcol[:, 0:1], 0.0,
                        OP.mult, OP.min,
                    )
                    e2 = tailp.tile([128, dout], F32)
                    nc.scalar.activation(e2, vm, AF.Exp, bias=zcol[:, 0:1])
                    ysb = tailp.tile([128, dout], F32)
                    nc.vector._custom_dve(
                        emx, out=ysb, in0=tp[:, 0:dout], in1=e2,
                        s0=rcol[:, 0:1], s1=0.0, imm2=0.0,
                    )
                    nc.sync.dma_start(
                        out=y_d[cc * 128 : (cc + 1) * 128, :], in_=ysb
                    )
    nc.compile()
    return nc


def _run(x, adj, w, a, a_coeff, b_coeff, c_coeff, d_coeff):
    global LAST_RESULTS, LAST_NC
    n, din = x.shape
    dout = w.shape[1]
    assert adj.shape == (n, n) and a.shape == (2 * dout, 1)
    rows = n // NCORES

    A = float(np.asarray(a_coeff).reshape(-1)[0])
    B = float(np.asarray(b_coeff).reshape(-1)[0])
    C = float(np.asarray(c_coeff).reshape(-1)[0])
    D0 = float(np.asarray(d_coeff).reshape(-1)[0])
    L1 = _leaky(A + B)
    assert C > 0.0, "kernel assumes c_coeff > 0"

    x = np.ascontiguousarray(x, dtype=np.float32)
    adj = np.asarray(adj, dtype=np.float32)
    assert ((adj == 0.0) | (adj == 1.0)).all(), "adj must be binary"
    w = np.ascontiguousarray(w, dtype=np.float32)
    a = np.ascontiguousarray(a, dtype=np.float32)

    h = x @ w
    hz1 = (h @ a[:dout, 0]).astype(np.float64)
    hz2 = (h @ a[dout:, 0]).astype(np.float64)

    k1 = L1 * C
    k2 = L1 * C * SLOPE
    d1 = L1 * D0
    d2 = L1 * SLOPE * D0
    s0 = k1 * float(hz2.max()) + d1 if k1 >= 0 else k1 * float(hz2.min()) + d1

    jperm = np.argsort(hz2, kind="stable")
    iperm = np.argsort(hz1, kind="stable")
    hz2_s = hz2[jperm]
    hz1_s = hz1[iperm]

    xt16 = np.ascontiguousarray(x.T).astype(np.float16)
    xt_j = np.ascontiguousarray(xt16[:, jperm])

    # per-group split points per core: y = C*(hz1_i + hz2_j) + D0 >= 0 is
    # branch-1; group g spans hz2_s[g*128 : (g+1)*128]
    ng = n // 128
    thr = -D0 / C
    consts_d = dict(
        k1=k1, k2=k2, b_f1=d1 - s0, b_rho=d2 - d1, sel_max=(L1 >= 0.0)
    )

    # Strided round-robin i-assignment: core c owns hz1-ranks c, c+8, ...
    # so every core's sorted-i column space samples hz1 near-identically and
    # one set of per-group column splits works for all cores.
    in_maps = []
    z_per_core = []
    for c in range(NCORES):
        iown = iperm[c::NCORES]
        h1own = hz1_s[c::NCORES]
        z2 = []
        z1 = []
        for g in range(ng):
            mn = hz2_s[g * 128]
            mx = hz2_s[(g + 1) * 128 - 1]
            # pure branch-2 (y<0 for all j in g): hz1_i < thr - mx
            # pure branch-1 (y>=0 for all j in g): hz1_i >= thr - mn
            lo = int(np.searchsorted(h1own, thr - mx, side="left"))
            hi = int(np.searchsorted(h1own, thr - mn, side="right"))
            z2.append(lo)
            z1.append(hi)
        z_per_core.append((z2, z1))
        adjt_enc = np.ascontiguousarray(
            adj[np.ix_(iown, jperm)].T
        ).astype(mybir.dt.np(F8E4))
        in_maps.append(
            {
                "adjt": adjt_enc,
                "xt": xt_j,
                "xt_own": np.ascontiguousarray(xt16[:, iown]),
                "w": w,
                "a": a,
            }
        )

    # all cores share one program: use the union splits (mixed region must
    # cover every core's mixed region; pure regions shrink to the common part)
    z2_u = [min(z_per_core[c][0][g] for c in range(NCORES)) for g in range(ng)]
    z1_u = [max(z_per_core[c][1][g] for c in range(NCORES)) for g in range(ng)]
    consts_d["z2"] = z2_u
    consts_d["z1"] = z1_u

    nc = _build(n, din, dout, rows, consts_d)
    LAST_NC = nc

    res = run_bass_kernel_spmd(
        nc, in_maps, core_ids=list(range(NCORES)), trace=TRACE
    )
    LAST_RESULTS = res
    y = np.empty((n, dout), dtype=np.float32)
    for c in range(NCORES):
        y[iperm[c::NCORES]] = res.results[c]["y"]
    return y


def kernel(x, adj, w, a, a_coeff, b_coeff, c_coeff, d_coeff):
    return _run(x, adj, w, a, a_coeff, b_coeff, c_coeff, d_coeff)


# revision 19
# speedup vs baseline: 1.7707x; 1.0665x over previous
"""DGAT attention head on 8 trn2 NeuronCores — separable-logit version.

Math: logit_ij = L1*leaky(C*(hz1_i + hz2_j) + D0) for adj_ji=1 (masked else),
with L1 = leaky(A+B) constant. Since leaky(y) = max(y, SLOPE*y) and exp is
monotone, the unnormalized softmax weight factorizes per branch:
  q_ji = adj_ji * sel(F1_j, R_i * F2_j)        sel = max if L1>=0 else min
  F1_j = exp(k1*hz2_j + d1 - s0), F2_j = exp(k2*hz2_j + d2 - s0),
  R_i  = exp((k2-k1)*hz1_i), k1 = L1*C, k2 = L1*C*SLOPE
(the per-query factor exp(k1*hz1_i + s0) cancels in the softmax ratio).

Host sorts j by hz2 (globally) and assigns core c the i's with hz1-rank in
[c*R, (c+1)*R) (sorted). Then per j-group g of 128 sorted j's the branch
boundary over sorted i-columns is monotone: columns [0, z2_g) are pure
branch-2, [z2_g, z1_g) mixed, [z1_g, 1024) pure branch-1. Pure ranges are
plain 0/1 fp8 adjacency matmuls against F-scaled bf16 stationaries:
  acc1 += adjT_g @ (F1 ⊙ [h|1])   (branch-1 cols)
  acc2 += adjT_g @ (F2 ⊙ [h|1])   (branch-2 cols; R_i applied at combine)
Mixed cols use moving q_b = adj ⊙ sel(1, R_i*rho_j), rho = F2/F1, against the
F1 stationary. Combine: hp = acc1 + R ⊙ acc2, then normalize + ELU as before.
Elementwise full-matrix work is gone; kernel is DMA/PE bound.
"""

import numpy as np

import concourse.bass as bass
import concourse.bacc as bacc
import concourse.mybir as mybir
import concourse.dve_ops as dve_ops
from concourse.dve_spec import Spec, Src0, Src1, maxx
from concourse.tile import TileContext
from concourse.bass_utils import run_bass_kernel_spmd

F32 = mybir.dt.float32
F16 = mybir.dt.float16
BF16 = mybir.dt.bfloat16
F8E4 = mybir.dt.float8e4
AF = mybir.ActivationFunctionType
OP = mybir.AluOpType

NCORES = 8
SLOPE = 0.2  # leakyrelu negative slope (fixed in the reference)

TRACE = False
LAST_RESULTS = None
LAST_NC = None


def _leaky(z):
    return z if z >= 0.0 else SLOPE * z


def _register_elu_max_op():
    name = "ELU_MAX_ANT"
    for op in dve_ops.OPS:
        if op.name == name:
            return op
    from concourse.dve_spec import C0, One

    spec = Spec(
        body=maxx(Src0 * C0, Src1 - One),
        reference=lambda in0, in1, s0, s1, imm2: np.maximum(
            in0 * s0, in1 - 1.0
        ).astype(np.float32),
    )
    return _finish_register(name, spec)


def _finish_register(name, spec):
    from concourse.dve_spec import lower
    from concourse.dve_ops import has_src1
    from concourse.dve_uop import DveOpSpec

    op = dve_ops.DveOp(name, spec, subdim=False, uops_sha={})
    dve_ops.OPS.append(op)
    dve_ops.CUSTOM_DVE_SPECS[name] = spec
    dve_ops._SUB_OPCODE_FOR_NAME[name] = (
        dve_ops._CUSTOM_DVE_ROW_BASE + len(dve_ops.OPS) - 1
    )
    assert dve_ops._SUB_OPCODE_FOR_NAME[name] < 0x20
    for ver in ("v3",):
        pinned = DveOpSpec(
            name=name,
            opcode=dve_ops.get_dve_sub_opcode(name),
            uops=lower(spec, ver=ver),
            rd1_en=has_src1(spec),
        ).sha(ver)
        op.uops_sha[ver] = pinned
        dve_ops._COMPILE_CACHE.pop((name, ver), None)
        op.compile(ver)
    return op


def _build(n, din, dout, rows, consts_d):
    """Build the SPMD Bass program (identical on all cores).

    consts_d: dict with floats k1, k2, b_f1 (=d1-s0), b_rho (=d2-d1),
    sel_max (bool), z2/z1 (per-group column split ints).
    """
    assert n % 512 == 0 and rows % 128 == 0 and din % 128 == 0
    ng = n // 128
    mt = 4
    nm = ng // mt
    kc = din // 128
    de = dout + 1      # [h | F] stationary width
    emx = _register_elu_max_op()

    k1 = consts_d["k1"]
    k2 = consts_d["k2"]
    b_f1 = consts_d["b_f1"]
    b_rho = consts_d["b_rho"]
    selop = OP.max if consts_d["sel_max"] else OP.min
    Z2 = consts_d["z2"]
    Z1 = consts_d["z1"]

    # last group index that emits a matmul into each accumulator (for stop=)
    last_g1 = max(
        (g for g in range(ng) if Z1[g] < rows or Z2[g] < Z1[g]), default=None
    )
    last_g2 = max((g for g in range(ng) if Z2[g] > 0), default=None)

    nc = bacc.Bacc("TRN2", target_bir_lowering=False)
    adjt_d = nc.dram_tensor("adjt", [n, rows], F8E4, kind="ExternalInput")
    xt_d = nc.dram_tensor("xt", [din, n], F16, kind="ExternalInput")
    xto_d = nc.dram_tensor("xt_own", [din, rows], F16, kind="ExternalInput")
    w_d = nc.dram_tensor("w", [din, dout], F32, kind="ExternalInput")
    a_d = nc.dram_tensor("a", [2 * dout, 1], F32, kind="ExternalInput")
    y_d = nc.dram_tensor("y", [rows, dout], F32, kind="ExternalOutput")

    with TileContext(nc) as tc:
        with (
            tc.tile_pool(name="consts", bufs=1) as consts,
            tc.tile_pool(name="adjp", bufs=16) as adjp,
            tc.tile_pool(name="bp", bufs=16) as bp,
            tc.tile_pool(name="tailp", bufs=16) as tailp,
        ):
            from concourse.masks import make_identity

            identity = consts.tile([128, 128], F32)
            make_identity(nc, identity)

            zcol = consts.tile([128, 1], F32)
            nc.vector.memset(zcol, 0.0)
            bf1col = consts.tile([128, 1], F32)
            nc.vector.memset(bf1col, b_f1)
            brhocol = consts.tile([128, 1], F32)
            nc.vector.memset(brhocol, b_rho)
            ones128 = consts.tile([128, 128], F16)
            nc.vector.memset(ones128, 1.0)
            zstat = consts.tile([128, de], BF16)
            nc.vector.memset(zstat, 0.0)
            zmov = consts.tile([128, rows], BF16)
            nc.gpsimd.memset(zmov, 0.0)

            # a1/a2 broadcast across partitions (partition-step-0 DMA)
            a_ap = a_d[:, :]
            a1bc = consts.tile([128, dout], F32)
            nc.sync.dma_start(
                out=a1bc,
                in_=bass.AP(tensor=a_ap.tensor, offset=0, ap=[[0, 128], [1, dout]]),
            )
            a2bc = consts.tile([128, dout], F32)
            nc.sync.dma_start(
                out=a2bc,
                in_=bass.AP(
                    tensor=a_ap.tensor, offset=dout, ap=[[0, 128], [1, dout]]
                ),
            )

            # wx_k = [w_k | w_k@a2] f16 stationaries for the h/hz2 compute;
            # wa1 column feeds the hz1 broadcast trick.
            wx = []
            wa1cols = []
            for k in range(kc):
                wxr = consts.tile([128, dout + 2], F32, name=f"wxr{k}")
                nc.sync.dma_start(
                    out=wxr[:, 0:dout], in_=w_d[k * 128 : (k + 1) * 128, :]
                )
                t1 = consts.tile([128, dout], F32, name=f"wa_t{k}")
                nc.vector.tensor_mul(t1, wxr[:, 0:dout], a1bc)
                nc.vector.reduce_sum(
                    wxr[:, dout : dout + 1], t1, axis=mybir.AxisListType.X
                )
                t2 = consts.tile([128, dout], F32, name=f"wb_t{k}")
                nc.vector.tensor_mul(t2, wxr[:, 0:dout], a2bc)
                nc.vector.reduce_sum(
                    wxr[:, dout + 1 : dout + 2], t2, axis=mybir.AxisListType.X
                )
                wxk = consts.tile([128, dout + 1], F16, name=f"wx{k}")
                nc.vector.tensor_copy(wxk[:, 0:dout], wxr[:, 0:dout])
                nc.vector.tensor_copy(
                    wxk[:, dout : dout + 1], wxr[:, dout + 1 : dout + 2]
                )
                wx.append(wxk)
                wa1cols.append(wxr)

            # stationaries and per-group columns
            h1s = consts.tile([128, ng, de], BF16)
            h2s = consts.tile([128, ng, de], BF16)
            f1cols = consts.tile([128, ng], F32)
            rhocols = consts.tile([128, ng], F32)
            Rbc = consts.tile([128, rows], BF16)
            hpT = consts.tile([de, rows], F32)

            with (
                tc.tile_pool(name="xtp", bufs=1) as xtp,
                tc.tile_pool(name="pshz", bufs=1, space="PSUM") as pshz,
                tc.tile_pool(name="pspre", bufs=2, space="PSUM") as pspre,
                tc.tile_pool(name="psacc", bufs=1, space="PSUM") as psacc,
            ):
                # --- own-x columns -> hz1 broadcast -> R broadcast ---
                xtos = []
                for k in range(kc):
                    xtok = xtp.tile([128, rows], F16, name=f"xto{k}")
                    nc.sync.dma_start(
                        out=xtok, in_=xto_d[k * 128 : (k + 1) * 128, :]
                    )
                    xtos.append(xtok)
                hz_ps = pshz.tile([128, rows], F32)
                for k in range(kc):
                    wa1bc = consts.tile([128, 128], F16, name=f"wa1bc{k}")
                    nc.vector.tensor_scalar_mul(
                        wa1bc, ones128, wa1cols[k][:, dout : dout + 1]
                    )
                    for n0 in range(0, rows, 512):
                        nc.tensor.matmul(
                            hz_ps[:, n0 : n0 + 512],
                            wa1bc,
                            xtos[k][:, n0 : n0 + 512],
                            start=(k == 0),
                            stop=(k == kc - 1),
                        )
                # R_i = exp((k2-k1)*hz1_i), broadcast on all partitions
                nc.scalar.activation(
                    Rbc, hz_ps, AF.Exp, bias=zcol[:, 0:1], scale=k2 - k1
                )

                # --- full x (j-sorted) -> per-group stationaries ---
                xchunk = 2048
                xts = [
                    xtp.tile([128, n], F16, name=f"xt{k}") for k in range(kc)
                ]
                for c0 in range(0, n, xchunk):
                    for k in range(kc):
                        nc.sync.dma_start(
                            out=xts[k][:, c0 : c0 + xchunk],
                            in_=xt_d[k * 128 : (k + 1) * 128, c0 : c0 + xchunk],
                        )

                grp = 4
                for g0 in range(0, ng, grp):
                    ps = pspre.tile([128, grp, de], F32, name="ps_h")
                    for gi in range(grp):
                        g = g0 + gi
                        for k in range(kc):
                            nc.tensor.matmul(
                                ps[:, gi, :],
                                xts[k][:, g * 128 : (g + 1) * 128],
                                wx[k],
                                start=(k == 0),
                                stop=(k == kc - 1),
                            )
                    # F1 into h1s[:, g, dout] (strided), rho into rhocols
                    nc.scalar.activation(
                        f1cols[:, g0 : g0 + grp],
                        ps[:, :, dout : dout + 1],
                        AF.Exp,
                        bias=bf1col[:, 0:1],
                        scale=k1,
                    )
                    nc.scalar.activation(
                        h1s[:, g0 : g0 + grp, dout : dout + 1],
                        ps[:, :, dout : dout + 1],
                        AF.Exp,
                        bias=bf1col[:, 0:1],
                        scale=k1,
                    )
                    nc.scalar.activation(
                        rhocols[:, g0 : g0 + grp],
                        ps[:, :, dout : dout + 1],
                        AF.Exp,
                        bias=brhocol[:, 0:1],
                        scale=k2 - k1,
                    )
                    for gi in range(grp):
                        g = g0 + gi
                        # h1s[:, g, :dout] = F1_j * h ; col dout already = F1_j
                        nc.scalar.activation(
                            h1s[:, g, 0:dout],
                            ps[:, gi, 0:dout],
                            AF.Copy,
                            bias=0.0,
                            scale=f1cols[:, g : g + 1],
                        )
                        # h2s = rho_j * h1s (including the F column)
                        nc.vector.tensor_scalar_mul(
                            h2s[:, g, :], h1s[:, g, :], rhocols[:, g : g + 1]
                        )

                # --- main loop: stream adjT megatiles, accumulate ---
                acc1 = psacc.tile([de, rows], F32)
                acc2 = psacc.tile([de, rows], F32)
                def acc_mm(acc, stat, mv, c0, c1, start, stop):
                    # matmul outputs may not cross the 512-col PSUM bank edge
                    for b0 in range(0, rows, 512):
                        lo = max(c0, b0)
                        hi = min(c1, b0 + 512)
                        if lo < hi:
                            nc.tensor.matmul(
                                acc[:, lo:hi],
                                stat,
                                mv[:, lo:hi],
                                start=start,
                                stop=stop,
                                skip_group_check=True,
                            )

                acc_mm(acc1, zstat, zmov, 0, rows, True, last_g1 is None)
                acc_mm(acc2, zstat, zmov, 0, rows, True, last_g2 is None)
                adjt_r = adjt_d[:, :].rearrange(
                    "(m t p) i -> m p t i", t=mt, p=128
                )
                for m in range(nm):
                    adjt_t = adjp.tile([128, mt, rows], F8E4)
                    nc.sync.dma_start(out=adjt_t, in_=adjt_r[m])
                    for t in range(mt):
                        g = mt * m + t
                        z2, z1 = Z2[g], Z1[g]
                        mv = adjt_t[:, t, :]
                        if z2 > 0:
                            acc_mm(
                                acc2, h2s[:, g, :], mv, 0, z2,
                                False, g == last_g2,
                            )
                        if z1 > z2:
                            # mixed columns: q_b = adj * sel(1, R_i*rho_j)
                            tb = bp.tile([128, z1 - z2], BF16, name="tb")
                            nc.vector.tensor_scalar(
                                tb,
                                Rbc[:, z2:z1],
                                rhocols[:, g : g + 1],
                                1.0,
                                OP.mult,
                                selop,
                            )
                            qb = bp.tile([128, z1 - z2], BF16, name="qb")
                            nc.vector.tensor_mul(qb, tb, mv[:, z2:z1])
                            for b0 in range(0, rows, 512):
                                lo = max(z2, b0)
                                hi = min(z1, b0 + 512)
                                if lo < hi:
                                    nc.tensor.matmul(
                                        acc1[:, lo:hi],
                                        h1s[:, g, :],
                                        qb[:, lo - z2 : hi - z2],
                                        start=False,
                                        stop=(
                                            g == last_g1 and z1 == rows
                                            and hi == z1
                                        ),
                                        skip_group_check=True,
                                    )
                        if z1 < rows:
                            acc_mm(
                                acc1, h1s[:, g, :], mv, z1, rows,
                                False, g == last_g1,
                            )

                # --- combine: hp = acc1 + R ⊙ acc2 (row dout = s) ---
                t2 = consts.tile([de, rows], F32, name="combine_t2")
                nc.vector.tensor_mul(t2, acc2, Rbc[0:de, :])
                nc.vector.tensor_add(hpT, t2, acc1)

            with tc.tile_pool(name="pstail", bufs=4, space="PSUM") as pstail:
                for cc in range(rows // 128):
                    tp = pstail.tile([128, de], F32)
                    nc.tensor.transpose(
                        tp,
                        hpT[:, cc * 128 : (cc + 1) * 128],
                        identity[0:de, 0:de],
                    )
                    rcol = tailp.tile([128, 1], F32)
                    nc.vector.reciprocal(rcol, tp[:, dout:de])
                    # elu(v) = max(v, exp(min(v, 0)) - 1), v = hp * (1/s)
                    vm = tailp.tile([128, dout], F32)
                    nc.vector.tensor_scalar(
                        vm, tp[:, 0:dout], rcol[:, 0:1], 0.0,
                        OP.mult, OP.min,
                    )
                    e2 = tailp.tile([128, dout], F32)
                    nc.scalar.activation(e2, vm, AF.Exp, bias=zcol[:, 0:1])
                    ysb = tailp.tile([128, dout], F32)
                    nc.vector._custom_dve(
                        emx, out=ysb, in0=tp[:, 0:dout], in1=e2,
                        s0=rcol[:, 0:1], s1=0.0, imm2=0.0,
                    )
                    nc.sync.dma_start(
                        out=y_d[cc * 128 : (cc + 1) * 128, :], in_=ysb
                    )
    nc.compile()
    return nc


def _run(x, adj, w, a, a_coeff, b_coeff, c_coeff, d_coeff):
    global LAST_RESULTS, LAST_NC
    n, din = x.shape
    dout = w.shape[1]
    assert adj.shape == (n, n) and a.shape == (2 * dout, 1)
    rows = n // NCORES

    A = float(np.asarray(a_coeff).reshape(-1)[0])
    B = float(np.asarray(b_coeff).reshape(-1)[0])
    C = float(np.asarray(c_coeff).reshape(-1)[0])
    D0 = float(np.asarray(d_coeff).reshape(-1)[0])
    L1 = _leaky(A + B)
    assert C > 0.0, "kernel assumes c_coeff > 0"

    x = np.ascontiguousarray(x, dtype=np.float32)
    adj = np.asarray(adj, dtype=np.float32)
    assert ((adj == 0.0) | (adj == 1.0)).all(), "adj must be binary"
    w = np.ascontiguousarray(w, dtype=np.float32)
    a = np.ascontiguousarray(a, dtype=np.float32)

    h = x @ w
    hz1 = (h @ a[:dout, 0]).astype(np.float64)
    hz2 = (h @ a[dout:, 0]).astype(np.float64)

    k1 = L1 * C
    k2 = L1 * C * SLOPE
    d1 = L1 * D0
    d2 = L1 * SLOPE * D0
    s0 = k1 * float(hz2.max()) + d1 if k1 >= 0 else k1 * float(hz2.min()) + d1

    jperm = np.argsort(hz2, kind="stable")
    iperm = np.argsort(hz1, kind="stable")
    hz2_s = hz2[jperm]
    hz1_s = hz1[iperm]

    xt16 = np.ascontiguousarray(x.T).astype(np.float16)
    xt_j = np.ascontiguousarray(xt16[:, jperm])

    # per-group split points per core: y = C*(hz1_i + hz2_j) + D0 >= 0 is
    # branch-1; group g spans hz2_s[g*128 : (g+1)*128]
    ng = n // 128
    thr = -D0 / C
    consts_d = dict(
        k1=k1, k2=k2, b_f1=d1 - s0, b_rho=d2 - d1, sel_max=(L1 >= 0.0)
    )

    # Strided round-robin i-assignment: core c owns hz1-ranks c, c+8, ...
    # so every core's sorted-i column space samples hz1 near-identically and
    # one set of per-group column splits works for all cores.
    in_maps = []
    z_per_core = []
    for c in range(NCORES):
        iown = iperm[c::NCORES]
        h1own = hz1_s[c::NCORES]
        z2 = []
        z1 = []
        for g in range(ng):
            mn = hz2_s[g * 128]
            mx = hz2_s[(g + 1) * 128 - 1]
            # pure branch-2 (y<0 for all j in g): hz1_i < thr - mx
            # pure branch-1 (y>=0 for all j in g): hz1_i >= thr - mn
            lo = int(np.searchsorted(h1own, thr - mx, side="left"))
            hi = int(np.searchsorted(h1own, thr - mn, side="right"))
            z2.append(lo)
            z1.append(hi)
        z_per_core.append((z2, z1))
        adjt_enc = np.ascontiguousarray(
            adj[np.ix_(iown, jperm)].T
        ).astype(mybir.dt.np(F8E4))
        in_maps.append(
            {
                "adjt": adjt_enc,
                "xt": xt_j,
                "xt_own": np.ascontiguousarray(xt16[:, iown]),
                "w": w,
                "a": a,
            }
        )

    # all cores share one program: use the union splits (mixed region must
    # cover every core's mixed region; pure regions shrink to the common part)
    z2_u = [min(z_per_core[c][0][g] for c in range(NCORES)) for g in range(ng)]
    z1_u = [max(z_per_core[c][1][g] for c in range(NCORES)) for g in range(ng)]
    consts_d["z2"] = z2_u
    consts_d["z1"] = z1_u

    nc = _build(n, din, dout, rows, consts_d)
    LAST_NC = nc

    res = run_bass_kernel_spmd(
        nc, in_maps, core_ids=list(range(NCORES)), trace=TRACE
    )
    LAST_RESULTS = res
    y = np.empty((n, dout), dtype=np.float32)
    for c in range(NCORES):
        y[iperm[c::NCORES]] = res.results[c]["y"]
    return y


def kernel(x, adj, w, a, a_coeff, b_coeff, c_coeff, d_coeff):
    return _run(x, adj, w, a, a_coeff, b_coeff, c_coeff, d_coeff)


# revision 23
# speedup vs baseline: 1.8194x; 1.0275x over previous
"""DGAT attention head on 8 trn2 NeuronCores — separable-logit version.

Math: logit_ij = L1*leaky(C*(hz1_i + hz2_j) + D0) for adj_ji=1 (masked else),
with L1 = leaky(A+B) constant. Since leaky(y) = max(y, SLOPE*y) and exp is
monotone, the unnormalized softmax weight factorizes per branch:
  q_ji = adj_ji * sel(F1_j, R_i * F2_j)        sel = max if L1>=0 else min
  F1_j = exp(k1*hz2_j + d1 - s0), F2_j = exp(k2*hz2_j + d2 - s0),
  R_i  = exp((k2-k1)*hz1_i), k1 = L1*C, k2 = L1*C*SLOPE
(the per-query factor exp(k1*hz1_i + s0) cancels in the softmax ratio).

Host sorts j by hz2 (globally) and assigns core c the i's with hz1-rank in
[c*R, (c+1)*R) (sorted). Then per j-group g of 128 sorted j's the branch
boundary over sorted i-columns is monotone: columns [0, z2_g) are pure
branch-2, [z2_g, z1_g) mixed, [z1_g, 1024) pure branch-1. Pure ranges are
plain 0/1 fp8 adjacency matmuls against F-scaled bf16 stationaries:
  acc1 += adjT_g @ (F1 ⊙ [h|1])   (branch-1 cols)
  acc2 += adjT_g @ (F2 ⊙ [h|1])   (branch-2 cols; R_i applied at combine)
Mixed cols use moving q_b = adj ⊙ sel(1, R_i*rho_j), rho = F2/F1, against the
F1 stationary. Combine: hp = acc1 + R ⊙ acc2, then normalize + ELU as before.
Elementwise full-matrix work is gone; kernel is DMA/PE bound.
"""

import numpy as np

import concourse.bass as bass
import concourse.bacc as bacc
import concourse.mybir as mybir
import concourse.dve_ops as dve_ops
from concourse.dve_spec import Spec, Src0, Src1, maxx
from concourse.tile import TileContext
from concourse.bass_utils import run_bass_kernel_spmd

F32 = mybir.dt.float32
F16 = mybir.dt.float16
BF16 = mybir.dt.bfloat16
F8E4 = mybir.dt.float8e4
AF = mybir.ActivationFunctionType
OP = mybir.AluOpType

NCORES = 8
SLOPE = 0.2  # leakyrelu negative slope (fixed in the reference)

TRACE = False
LAST_RESULTS = None
LAST_NC = None


def _leaky(z):
    return z if z >= 0.0 else SLOPE * z


def _register_elu_max_op():
    name = "ELU_MAX_ANT"
    for op in dve_ops.OPS:
        if op.name == name:
            return op
    from concourse.dve_spec import C0, One

    spec = Spec(
        body=maxx(Src0 * C0, Src1 - One),
        reference=lambda in0, in1, s0, s1, imm2: np.maximum(
            in0 * s0, in1 - 1.0
        ).astype(np.float32),
    )
    return _finish_register(name, spec)


def _finish_register(name, spec):
    from concourse.dve_spec import lower
    from concourse.dve_ops import has_src1
    from concourse.dve_uop import DveOpSpec

    op = dve_ops.DveOp(name, spec, subdim=False, uops_sha={})
    dve_ops.OPS.append(op)
    dve_ops.CUSTOM_DVE_SPECS[name] = spec
    dve_ops._SUB_OPCODE_FOR_NAME[name] = (
        dve_ops._CUSTOM_DVE_ROW_BASE + len(dve_ops.OPS) - 1
    )
    assert dve_ops._SUB_OPCODE_FOR_NAME[name] < 0x20
    for ver in ("v3",):
        pinned = DveOpSpec(
            name=name,
            opcode=dve_ops.get_dve_sub_opcode(name),
            uops=lower(spec, ver=ver),
            rd1_en=has_src1(spec),
        ).sha(ver)
        op.uops_sha[ver] = pinned
        dve_ops._COMPILE_CACHE.pop((name, ver), None)
        op.compile(ver)
    return op


def _build(n, din, dout, rows, consts_d):
    """Build the SPMD Bass program (identical on all cores).

    consts_d: dict with floats k1, k2, b_f1 (=d1-s0), b_rho (=d2-d1),
    sel_max (bool), z2/z1 (per-group column split ints).
    """
    assert n % 512 == 0 and rows % 128 == 0 and din % 128 == 0
    ng = n // 128
    mt = 4
    nm = ng // mt
    kc = din // 128
    de = dout + 1      # [h | F] stationary width
    emx = _register_elu_max_op()

    k1 = consts_d["k1"]
    k2 = consts_d["k2"]
    b_f1 = consts_d["b_f1"]
    b_rho = consts_d["b_rho"]
    selop = OP.max if consts_d["sel_max"] else OP.min
    Z2 = consts_d["z2"]
    Z1 = consts_d["z1"]

    # last group index that emits a matmul into each accumulator (for stop=)
    last_g1 = max(
        (g for g in range(ng) if Z1[g] < rows or Z2[g] < Z1[g]), default=None
    )
    last_g2 = max((g for g in range(ng) if Z2[g] > 0), default=None)

    nc = bacc.Bacc("TRN2", target_bir_lowering=False)
    adjt_d = nc.dram_tensor("adjt", [n, rows], F8E4, kind="ExternalInput")
    xt_d = nc.dram_tensor("xt", [din, n], F16, kind="ExternalInput")
    xto_d = nc.dram_tensor("xt_own", [din, rows], F16, kind="ExternalInput")
    w_d = nc.dram_tensor("w", [din, dout], F32, kind="ExternalInput")
    a_d = nc.dram_tensor("a", [2 * dout, 1], F32, kind="ExternalInput")
    y_d = nc.dram_tensor("y", [rows, dout], F32, kind="ExternalOutput")

    with TileContext(nc) as tc:
        with (
            tc.tile_pool(name="consts", bufs=1) as consts,
            tc.tile_pool(name="adjp", bufs=16) as adjp,
            tc.tile_pool(name="bp", bufs=16) as bp,
            tc.tile_pool(name="tailp", bufs=16) as tailp,
        ):
            from concourse.masks import make_identity

            identity = consts.tile([128, 128], F32)
            make_identity(nc, identity)

            zcol = consts.tile([128, 1], F32)
            nc.vector.memset(zcol, 0.0)
            bf1col = consts.tile([128, 1], F32)
            nc.vector.memset(bf1col, b_f1)
            brhocol = consts.tile([128, 1], F32)
            nc.vector.memset(brhocol, b_rho)
            ones128 = consts.tile([128, 128], F16)
            nc.vector.memset(ones128, 1.0)
            zstat = consts.tile([128, de], BF16)
            nc.vector.memset(zstat, 0.0)
            zmov = consts.tile([128, rows], BF16)
            nc.gpsimd.memset(zmov, 0.0)

            # a1/a2 broadcast across partitions (partition-step-0 DMA)
            a_ap = a_d[:, :]
            a1bc = consts.tile([128, dout], F32)
            nc.sync.dma_start(
                out=a1bc,
                in_=bass.AP(tensor=a_ap.tensor, offset=0, ap=[[0, 128], [1, dout]]),
            )
            a2bc = consts.tile([128, dout], F32)
            nc.sync.dma_start(
                out=a2bc,
                in_=bass.AP(
                    tensor=a_ap.tensor, offset=dout, ap=[[0, 128], [1, dout]]
                ),
            )

            # wx_k = [w_k | w_k@a2] f16 stationaries for the h/hz2 compute;
            # wa1 column feeds the hz1 broadcast trick.
            wx = []
            wa1cols = []
            for k in range(kc):
                wxr = consts.tile([128, dout + 2], F32, name=f"wxr{k}")
                nc.sync.dma_start(
                    out=wxr[:, 0:dout], in_=w_d[k * 128 : (k + 1) * 128, :]
                )
                t1 = consts.tile([128, dout], F32, name=f"wa_t{k}")
                nc.vector.tensor_mul(t1, wxr[:, 0:dout], a1bc)
                nc.vector.reduce_sum(
                    wxr[:, dout : dout + 1], t1, axis=mybir.AxisListType.X
                )
                t2 = consts.tile([128, dout], F32, name=f"wb_t{k}")
                nc.vector.tensor_mul(t2, wxr[:, 0:dout], a2bc)
                nc.vector.reduce_sum(
                    wxr[:, dout + 1 : dout + 2], t2, axis=mybir.AxisListType.X
                )
                wxk = consts.tile([128, dout + 1], F16, name=f"wx{k}")
                nc.vector.tensor_copy(wxk[:, 0:dout], wxr[:, 0:dout])
                nc.vector.tensor_copy(
                    wxk[:, dout : dout + 1], wxr[:, dout + 1 : dout + 2]
                )
                wx.append(wxk)
                wa1cols.append(wxr)

            # stationaries and per-group columns
            h1s = consts.tile([128, ng, de], BF16)
            h2s = consts.tile([128, ng, de], BF16)
            f1cols = consts.tile([128, ng], F32)
            rhocols = consts.tile([128, ng], F32)
            Rbc = consts.tile([128, rows], BF16)
            hpT = consts.tile([de, rows], F32)

            with (
                tc.tile_pool(name="xtp", bufs=1) as xtp,
                tc.tile_pool(name="pshz", bufs=1, space="PSUM") as pshz,
                tc.tile_pool(name="pspre", bufs=2, space="PSUM") as pspre,
                tc.tile_pool(name="psacc", bufs=1, space="PSUM") as psacc,
            ):
                # --- own-x columns -> hz1 broadcast -> R broadcast ---
                xtos = []
                for k in range(kc):
                    xtok = xtp.tile([128, rows], F16, name=f"xto{k}")
                    nc.sync.dma_start(
                        out=xtok, in_=xto_d[k * 128 : (k + 1) * 128, :]
                    )
                    xtos.append(xtok)
                hz_ps = pshz.tile([128, rows], F32)
                for k in range(kc):
                    wa1bc = consts.tile([128, 128], F16, name=f"wa1bc{k}")
                    nc.vector.tensor_scalar_mul(
                        wa1bc, ones128, wa1cols[k][:, dout : dout + 1]
                    )
                    for n0 in range(0, rows, 512):
                        nc.tensor.matmul(
                            hz_ps[:, n0 : n0 + 512],
                            wa1bc,
                            xtos[k][:, n0 : n0 + 512],
                            start=(k == 0),
                            stop=(k == kc - 1),
                        )
                # R_i = exp((k2-k1)*hz1_i), broadcast on all partitions
                nc.scalar.activation(
                    Rbc, hz_ps, AF.Exp, bias=zcol[:, 0:1], scale=k2 - k1
                )

                # --- full x (j-sorted): DMA interleaved with adjT megatiles
                # so accumulation matmuls can start as early as possible ---
                xchunk = 2048
                xts = [
                    xtp.tile([128, n], F16, name=f"xt{k}") for k in range(kc)
                ]
                adjt_r = adjt_d[:, :].rearrange(
                    "(m t p) i -> m p t i", t=mt, p=128
                )
                adjt_tiles = [None] * nm

                def issue_xt(ci):
                    c0 = ci * xchunk
                    for k in range(kc):
                        nc.sync.dma_start(
                            out=xts[k][:, c0 : c0 + xchunk],
                            in_=xt_d[k * 128 : (k + 1) * 128, c0 : c0 + xchunk],
                        )

                def issue_adjt(m):
                    adjt_tiles[m] = adjp.tile(
                        [128, mt, rows], F8E4, name="adjt"
                    )
                    nc.sync.dma_start(out=adjt_tiles[m], in_=adjt_r[m])

                ncp = n // xchunk  # xt chunk pairs
                mt_per_cp = nm // ncp
                issue_xt(0)
                issue_xt(1)
                for ci in range(ncp):
                    if ci >= 2:
                        issue_xt(ci)
                    for m in range(ci * mt_per_cp, (ci + 1) * mt_per_cp):
                        issue_adjt(m)

                grp = 4
                for g0 in range(0, ng, grp):
                    sl = slice(g0, g0 + grp)
                    ps = pspre.tile([128, grp, de], F32, name="ps_h")
                    for gi in range(grp):
                        g = g0 + gi
                        for k in range(kc):
                            nc.tensor.matmul(
                                ps[:, gi, :],
                                xts[k][:, g * 128 : (g + 1) * 128],
                                wx[k],
                                start=(k == 0),
                                stop=(k == kc - 1),
                            )
                    nc.scalar.activation(
                        f1cols[:, sl],
                        ps[:, :, dout : dout + 1],
                        AF.Exp,
                        bias=bf1col[:, 0:1],
                        scale=k1,
                    )
                    nc.scalar.activation(
                        rhocols[:, sl],
                        ps[:, :, dout : dout + 1],
                        AF.Exp,
                        bias=brhocol[:, 0:1],
                        scale=k2 - k1,
                    )
                    # h1s = F1 ⊙ ps (col dout garbage, fixed by the exp below)
                    nc.vector.tensor_mul(
                        h1s[:, sl, :],
                        ps,
                        f1cols[:, sl, None].broadcast_to([128, grp, de]),
                    )
                    nc.scalar.activation(
                        h1s[:, sl, dout : dout + 1],
                        ps[:, :, dout : dout + 1],
                        AF.Exp,
                        bias=bf1col[:, 0:1],
                        scale=k1,
                    )
                    # h2s = rho ⊙ h1s (col dout -> rho*F1 = F2) on gpsimd
                    nc.gpsimd.tensor_mul(
                        h2s[:, sl, :],
                        h1s[:, sl, :],
                        rhocols[:, sl, None].broadcast_to([128, grp, de]),
                    )

                # --- main loop: accumulate over the resident adjT tiles ---
                acc1 = psacc.tile([de, rows], F32)
                acc2 = psacc.tile([de, rows], F32)
                def acc_mm(acc, stat, mv, c0, c1, start, stop):
                    # matmul outputs may not cross the 512-col PSUM bank edge
                    for b0 in range(0, rows, 512):
                        lo = max(c0, b0)
                        hi = min(c1, b0 + 512)
                        if lo < hi:
                            nc.tensor.matmul(
                                acc[:, lo:hi],
                                stat,
                                mv[:, lo:hi],
                                start=start,
                                stop=stop,
                                skip_group_check=True,
                            )

                acc_mm(acc1, zstat, zmov, 0, rows, True, last_g1 is None)
                acc_mm(acc2, zstat, zmov, 0, rows, True, last_g2 is None)
                for m in range(nm):
                    adjt_t = adjt_tiles[m]
                    for t in range(mt):
                        g = mt * m + t
                        z2, z1 = Z2[g], Z1[g]
                        mv = adjt_t[:, t, :]
                        if z2 > 0:
                            acc_mm(
                                acc2, h2s[:, g, :], mv, 0, z2,
                                False, g == last_g2,
                            )
                        if z1 > z2:
                            # mixed columns: q_b = adj * sel(1, R_i*rho_j)
                            tb = bp.tile([128, z1 - z2], BF16, name="tb")
                            nc.vector.tensor_scalar(
                                tb,
                                Rbc[:, z2:z1],
                                rhocols[:, g : g + 1],
                                1.0,
                                OP.mult,
                                selop,
                            )
                            qb = bp.tile([128, z1 - z2], BF16, name="qb")
                            nc.vector.tensor_mul(qb, tb, mv[:, z2:z1])
                            for b0 in range(0, rows, 512):
                                lo = max(z2, b0)
                                hi = min(z1, b0 + 512)
                                if lo < hi:
                                    nc.tensor.matmul(
                                        acc1[:, lo:hi],
                                        h1s[:, g, :],
                                        qb[:, lo - z2 : hi - z2],
                                        start=False,
                                        stop=(
                                            g == last_g1 and z1 == rows
                                            and hi == z1
                                        ),
                                        skip_group_check=True,
                                    )
                        if z1 < rows:
                            acc_mm(
                                acc1, h1s[:, g, :], mv, z1, rows,
                                False, g == last_g1,
                            )

                # --- combine: hp = acc1 + R ⊙ acc2 (row dout = s) ---
                t2 = consts.tile([de, rows], F32, name="combine_t2")
                nc.vector.tensor_mul(t2, acc2, Rbc[0:de, :])
                nc.vector.tensor_add(hpT, t2, acc1)

            with tc.tile_pool(name="pstail", bufs=4, space="PSUM") as pstail:
                for cc in range(rows // 128):
                    tp = pstail.tile([128, de], F32)
                    nc.tensor.transpose(
                        tp,
                        hpT[:, cc * 128 : (cc + 1) * 128],
                        identity[0:de, 0:de],
                    )
                    rcol = tailp.tile([128, 1], F32)
                    nc.vector.reciprocal(rcol, tp[:, dout:de])
                    # elu(v) = max(v, exp(min(v, 0)) - 1), v = hp * (1/s)
                    vm = tailp.tile([128, dout], F32)
                    nc.vector.tensor_scalar(
                        vm, tp[:, 0:dout], rcol[:, 0:1], 0.0,
                        OP.mult, OP.min,
                    )
                    e2 = tailp.tile([128, dout], F32)
                    nc.scalar.activation(e2, vm, AF.Exp, bias=zcol[:, 0:1])
                    ysb = tailp.tile([128, dout], F32)
                    nc.vector._custom_dve(
                        emx, out=ysb, in0=tp[:, 0:dout], in1=e2,
                        s0=rcol[:, 0:1], s1=0.0, imm2=0.0,
                    )
                    nc.sync.dma_start(
                        out=y_d[cc * 128 : (cc + 1) * 128, :], in_=ysb
                    )
    nc.compile()
    return nc


def _run(x, adj, w, a, a_coeff, b_coeff, c_coeff, d_coeff):
    global LAST_RESULTS, LAST_NC
    n, din = x.shape
    dout = w.shape[1]
    assert adj.shape == (n, n) and a.shape == (2 * dout, 1)
    rows = n // NCORES

    A = float(np.asarray(a_coeff).reshape(-1)[0])
    B = float(np.asarray(b_coeff).reshape(-1)[0])
    C = float(np.asarray(c_coeff).reshape(-1)[0])
    D0 = float(np.asarray(d_coeff).reshape(-1)[0])
    L1 = _leaky(A + B)
    assert C > 0.0, "kernel assumes c_coeff > 0"

    x = np.ascontiguousarray(x, dtype=np.float32)
    adj = np.asarray(adj, dtype=np.float32)
    assert ((adj == 0.0) | (adj == 1.0)).all(), "adj must be binary"
    w = np.ascontiguousarray(w, dtype=np.float32)
    a = np.ascontiguousarray(a, dtype=np.float32)

    h = x @ w
    hz1 = (h @ a[:dout, 0]).astype(np.float64)
    hz2 = (h @ a[dout:, 0]).astype(np.float64)

    k1 = L1 * C
    k2 = L1 * C * SLOPE
    d1 = L1 * D0
    d2 = L1 * SLOPE * D0
    s0 = k1 * float(hz2.max()) + d1 if k1 >= 0 else k1 * float(hz2.min()) + d1

    jperm = np.argsort(hz2, kind="stable")
    iperm = np.argsort(hz1, kind="stable")
    hz2_s = hz2[jperm]
    hz1_s = hz1[iperm]

    xt16 = np.ascontiguousarray(x.T).astype(np.float16)
    xt_j = np.ascontiguousarray(xt16[:, jperm])

    # per-group split points per core: y = C*(hz1_i + hz2_j) + D0 >= 0 is
    # branch-1; group g spans hz2_s[g*128 : (g+1)*128]
    ng = n // 128
    thr = -D0 / C
    consts_d = dict(
        k1=k1, k2=k2, b_f1=d1 - s0, b_rho=d2 - d1, sel_max=(L1 >= 0.0)
    )

    # Strided round-robin i-assignment: core c owns hz1-ranks c, c+8, ...
    # so every core's sorted-i column space samples hz1 near-identically and
    # one set of per-group column splits works for all cores.
    in_maps = []
    z_per_core = []
    for c in range(NCORES):
        iown = iperm[c::NCORES]
        h1own = hz1_s[c::NCORES]
        z2 = []
        z1 = []
        for g in range(ng):
            mn = hz2_s[g * 128]
            mx = hz2_s[(g + 1) * 128 - 1]
            # pure branch-2 (y<0 for all j in g): hz1_i < thr - mx
            # pure branch-1 (y>=0 for all j in g): hz1_i >= thr - mn
            lo = int(np.searchsorted(h1own, thr - mx, side="left"))
            hi = int(np.searchsorted(h1own, thr - mn, side="right"))
            z2.append(lo)
            z1.append(hi)
        z_per_core.append((z2, z1))
        adjt_enc = np.ascontiguousarray(
            adj[np.ix_(iown, jperm)].T
        ).astype(mybir.dt.np(F8E4))
        in_maps.append(
            {
                "adjt": adjt_enc,
                "xt": xt_j,
                "xt_own": np.ascontiguousarray(xt16[:, iown]),
                "w": w,
                "a": a,
            }
        )

    # all cores share one program: use the union splits (mixed region must
    # cover every core's mixed region; pure regions shrink to the common part)
    z2_u = [min(z_per_core[c][0][g] for c in range(NCORES)) for g in range(ng)]
    z1_u = [max(z_per_core[c][1][g] for c in range(NCORES)) for g in range(ng)]
    consts_d["z2"] = z2_u
    consts_d["z1"] = z1_u

    nc = _build(n, din, dout, rows, consts_d)
    LAST_NC = nc

    res = run_bass_kernel_spmd(
        nc, in_maps, core_ids=list(range(NCORES)), trace=TRACE
    )
    LAST_RESULTS = res
    y = np.empty((n, dout), dtype=np.float32)
    for c in range(NCORES):
        y[iperm[c::NCORES]] = res.results[c]["y"]
    return y


def kernel(x, adj, w, a, a_coeff, b_coeff, c_coeff, d_coeff):
    return _run(x, adj, w, a, a_coeff, b_coeff, c_coeff, d_coeff)


# revision 25
# speedup vs baseline: 1.8350x; 1.0085x over previous
"""DGAT attention head on 8 trn2 NeuronCores — separable-logit version.

Math: logit_ij = L1*leaky(C*(hz1_i + hz2_j) + D0) for adj_ji=1 (masked else),
with L1 = leaky(A+B) constant. Since leaky(y) = max(y, SLOPE*y) and exp is
monotone, the unnormalized softmax weight factorizes per branch:
  q_ji = adj_ji * sel(F1_j, R_i * F2_j)        sel = max if L1>=0 else min
  F1_j = exp(k1*hz2_j + d1 - s0), F2_j = exp(k2*hz2_j + d2 - s0),
  R_i  = exp((k2-k1)*hz1_i), k1 = L1*C, k2 = L1*C*SLOPE
(the per-query factor exp(k1*hz1_i + s0) cancels in the softmax ratio).

Host sorts j by hz2 (globally) and assigns core c the i's with hz1-rank in
[c*R, (c+1)*R) (sorted). Then per j-group g of 128 sorted j's the branch
boundary over sorted i-columns is monotone: columns [0, z2_g) are pure
branch-2, [z2_g, z1_g) mixed, [z1_g, 1024) pure branch-1. Pure ranges are
plain 0/1 fp8 adjacency matmuls against F-scaled bf16 stationaries:
  acc1 += adjT_g @ (F1 ⊙ [h|1])   (branch-1 cols)
  acc2 += adjT_g @ (F2 ⊙ [h|1])   (branch-2 cols; R_i applied at combine)
Mixed cols use moving q_b = adj ⊙ sel(1, R_i*rho_j), rho = F2/F1, against the
F1 stationary. Combine: hp = acc1 + R ⊙ acc2, then normalize + ELU as before.
Elementwise full-matrix work is gone; kernel is DMA/PE bound.
"""

import numpy as np

import concourse.bass as bass
import concourse.bacc as bacc
import concourse.mybir as mybir
import concourse.dve_ops as dve_ops
from concourse.dve_spec import Spec, Src0, Src1, maxx
from concourse.tile import TileContext
from concourse.bass_utils import run_bass_kernel_spmd

F32 = mybir.dt.float32
F16 = mybir.dt.float16
BF16 = mybir.dt.bfloat16
F8E4 = mybir.dt.float8e4
AF = mybir.ActivationFunctionType
OP = mybir.AluOpType

NCORES = 8
SLOPE = 0.2  # leakyrelu negative slope (fixed in the reference)

TRACE = False
LAST_RESULTS = None
LAST_NC = None


def _leaky(z):
    return z if z >= 0.0 else SLOPE * z


def _register_elu_max_op():
    name = "ELU_MAX_ANT"
    for op in dve_ops.OPS:
        if op.name == name:
            return op
    from concourse.dve_spec import C0, One

    spec = Spec(
        body=maxx(Src0 * C0, Src1 - One),
        reference=lambda in0, in1, s0, s1, imm2: np.maximum(
            in0 * s0, in1 - 1.0
        ).astype(np.float32),
    )
    return _finish_register(name, spec)


def _finish_register(name, spec):
    from concourse.dve_spec import lower
    from concourse.dve_ops import has_src1
    from concourse.dve_uop import DveOpSpec

    op = dve_ops.DveOp(name, spec, subdim=False, uops_sha={})
    dve_ops.OPS.append(op)
    dve_ops.CUSTOM_DVE_SPECS[name] = spec
    dve_ops._SUB_OPCODE_FOR_NAME[name] = (
        dve_ops._CUSTOM_DVE_ROW_BASE + len(dve_ops.OPS) - 1
    )
    assert dve_ops._SUB_OPCODE_FOR_NAME[name] < 0x20
    for ver in ("v3",):
        pinned = DveOpSpec(
            name=name,
            opcode=dve_ops.get_dve_sub_opcode(name),
            uops=lower(spec, ver=ver),
            rd1_en=has_src1(spec),
        ).sha(ver)
        op.uops_sha[ver] = pinned
        dve_ops._COMPILE_CACHE.pop((name, ver), None)
        op.compile(ver)
    return op


def _build(n, din, dout, rows, consts_d):
    """Build the SPMD Bass program (identical on all cores).

    consts_d: dict with floats k1, k2, b_f1 (=d1-s0), b_rho (=d2-d1),
    sel_max (bool), z2/z1 (per-group column split ints).
    """
    assert n % 512 == 0 and rows % 128 == 0 and din % 128 == 0
    ng = n // 128
    mt = 4
    nm = ng // mt
    kc = din // 128
    de = dout + 1      # [h | F] stationary width
    emx = _register_elu_max_op()

    k1 = consts_d["k1"]
    k2 = consts_d["k2"]
    b_f1 = consts_d["b_f1"]
    b_rho = consts_d["b_rho"]
    selop = OP.max if consts_d["sel_max"] else OP.min
    Z2 = consts_d["z2"]
    Z1 = consts_d["z1"]

    # last group index that emits a matmul into each accumulator (for stop=)
    last_g1 = max(
        (g for g in range(ng) if Z1[g] < rows or Z2[g] < Z1[g]), default=None
    )
    last_g2 = max((g for g in range(ng) if Z2[g] > 0), default=None)

    nc = bacc.Bacc("TRN2", target_bir_lowering=False)
    adjt_d = nc.dram_tensor("adjt", [n, rows], F8E4, kind="ExternalInput")
    xt_d = nc.dram_tensor("xt", [din, n], F16, kind="ExternalInput")
    xto_d = nc.dram_tensor("xt_own", [din, rows], F16, kind="ExternalInput")
    w_d = nc.dram_tensor("w", [din, dout], F32, kind="ExternalInput")
    a_d = nc.dram_tensor("a", [2 * dout, 1], F32, kind="ExternalInput")
    y_d = nc.dram_tensor("y", [rows, dout], F32, kind="ExternalOutput")

    with TileContext(nc) as tc:
        with (
            tc.tile_pool(name="consts", bufs=1) as consts,
            tc.tile_pool(name="adjp", bufs=16) as adjp,
            tc.tile_pool(name="bp", bufs=16) as bp,
            tc.tile_pool(name="tailp", bufs=16) as tailp,
        ):
            from concourse.masks import make_identity

            identity = consts.tile([128, 128], F32)
            make_identity(nc, identity)

            zcol = consts.tile([128, 1], F32)
            nc.vector.memset(zcol, 0.0)
            bf1col = consts.tile([128, 1], F32)
            nc.vector.memset(bf1col, b_f1)
            brhocol = consts.tile([128, 1], F32)
            nc.vector.memset(brhocol, b_rho)
            ones128 = consts.tile([128, 128], F16)
            nc.vector.memset(ones128, 1.0)
            zstat = consts.tile([128, de], BF16)
            nc.vector.memset(zstat, 0.0)
            zmov = consts.tile([128, rows], BF16)
            nc.gpsimd.memset(zmov, 0.0)

            # a1/a2 broadcast across partitions (partition-step-0 DMA)
            a_ap = a_d[:, :]
            a1bc = consts.tile([128, dout], F32)
            nc.sync.dma_start(
                out=a1bc,
                in_=bass.AP(tensor=a_ap.tensor, offset=0, ap=[[0, 128], [1, dout]]),
            )
            a2bc = consts.tile([128, dout], F32)
            nc.sync.dma_start(
                out=a2bc,
                in_=bass.AP(
                    tensor=a_ap.tensor, offset=dout, ap=[[0, 128], [1, dout]]
                ),
            )

            # wx_k = [w_k | w_k@a2] f16 stationaries for the h/hz2 compute;
            # wa1 column feeds the hz1 broadcast trick.
            wx = []
            wa1cols = []
            for k in range(kc):
                wxr = consts.tile([128, dout + 2], F32, name=f"wxr{k}")
                nc.sync.dma_start(
                    out=wxr[:, 0:dout], in_=w_d[k * 128 : (k + 1) * 128, :]
                )
                t1 = consts.tile([128, dout], F32, name=f"wa_t{k}")
                nc.vector.tensor_mul(t1, wxr[:, 0:dout], a1bc)
                nc.vector.reduce_sum(
                    wxr[:, dout : dout + 1], t1, axis=mybir.AxisListType.X
                )
                t2 = consts.tile([128, dout], F32, name=f"wb_t{k}")
                nc.vector.tensor_mul(t2, wxr[:, 0:dout], a2bc)
                nc.vector.reduce_sum(
                    wxr[:, dout + 1 : dout + 2], t2, axis=mybir.AxisListType.X
                )
                wxk = consts.tile([128, dout + 1], F16, name=f"wx{k}")
                nc.vector.tensor_copy(wxk[:, 0:dout], wxr[:, 0:dout])
                nc.vector.tensor_copy(
                    wxk[:, dout : dout + 1], wxr[:, dout + 1 : dout + 2]
                )
                wx.append(wxk)
                wa1cols.append(wxr)

            # stationaries and per-group columns
            h1s = consts.tile([128, ng, de], BF16)
            h2s = consts.tile([128, ng, de], BF16)
            f1cols = consts.tile([128, ng], F32)
            rhocols = consts.tile([128, ng], F32)
            Rbc = consts.tile([128, rows], BF16)
            hpT = consts.tile([de, rows], F32)

            with (
                tc.tile_pool(name="xtp", bufs=1) as xtp,
                tc.tile_pool(name="pshz", bufs=1, space="PSUM") as pshz,
                tc.tile_pool(name="pspre", bufs=2, space="PSUM") as pspre,
                tc.tile_pool(name="psacc", bufs=1, space="PSUM") as psacc,
            ):
                # --- own-x columns -> hz1 broadcast -> R broadcast ---
                xtos = []
                for k in range(kc):
                    xtok = xtp.tile([128, rows], F16, name=f"xto{k}")
                    nc.sync.dma_start(
                        out=xtok, in_=xto_d[k * 128 : (k + 1) * 128, :]
                    )
                    xtos.append(xtok)
                hz_ps = pshz.tile([128, rows], F32)
                for k in range(kc):
                    wa1bc = consts.tile([128, 128], F16, name=f"wa1bc{k}")
                    nc.vector.tensor_scalar_mul(
                        wa1bc, ones128, wa1cols[k][:, dout : dout + 1]
                    )
                    for n0 in range(0, rows, 512):
                        nc.tensor.matmul(
                            hz_ps[:, n0 : n0 + 512],
                            wa1bc,
                            xtos[k][:, n0 : n0 + 512],
                            start=(k == 0),
                            stop=(k == kc - 1),
                        )
                # R_i = exp((k2-k1)*hz1_i), broadcast on all partitions
                nc.scalar.activation(
                    Rbc, hz_ps, AF.Exp, bias=zcol[:, 0:1], scale=k2 - k1
                )

                # --- full x (j-sorted): DMA interleaved with adjT megatiles
                # so accumulation matmuls can start as early as possible ---
                xchunk = 2048
                xts = [
                    xtp.tile([128, n], F16, name=f"xt{k}") for k in range(kc)
                ]
                adjt_r = adjt_d[:, :].rearrange(
                    "(m t p) i -> m p t i", t=mt, p=128
                )
                adjt_tiles = [None] * nm

                def issue_xt(ci):
                    c0 = ci * xchunk
                    for k in range(kc):
                        nc.sync.dma_start(
                            out=xts[k][:, c0 : c0 + xchunk],
                            in_=xt_d[k * 128 : (k + 1) * 128, c0 : c0 + xchunk],
                        )

                def issue_adjt(m):
                    adjt_tiles[m] = adjp.tile(
                        [128, mt, rows], F8E4, name="adjt"
                    )
                    nc.sync.dma_start(out=adjt_tiles[m], in_=adjt_r[m])

                ncp = n // xchunk  # xt chunk pairs
                mt_per_cp = nm // ncp
                issue_xt(0)
                issue_adjt(0)
                issue_xt(1)
                for ci in range(ncp):
                    if ci >= 2:
                        issue_xt(ci)
                    for m in range(ci * mt_per_cp, (ci + 1) * mt_per_cp):
                        if m > 0:
                            issue_adjt(m)

                grp = 4
                for g0 in range(0, ng, grp):
                    sl = slice(g0, g0 + grp)
                    ps = pspre.tile([128, grp, de], F32, name="ps_h")
                    for gi in range(grp):
                        g = g0 + gi
                        for k in range(kc):
                            nc.tensor.matmul(
                                ps[:, gi, :],
                                xts[k][:, g * 128 : (g + 1) * 128],
                                wx[k],
                                start=(k == 0),
                                stop=(k == kc - 1),
                            )
                    nc.scalar.activation(
                        f1cols[:, sl],
                        ps[:, :, dout : dout + 1],
                        AF.Exp,
                        bias=bf1col[:, 0:1],
                        scale=k1,
                    )
                    nc.scalar.activation(
                        rhocols[:, sl],
                        ps[:, :, dout : dout + 1],
                        AF.Exp,
                        bias=brhocol[:, 0:1],
                        scale=k2 - k1,
                    )
                    # h1s = F1 ⊙ ps (col dout garbage, fixed by the exp below)
                    nc.vector.tensor_mul(
                        h1s[:, sl, :],
                        ps,
                        f1cols[:, sl, None].broadcast_to([128, grp, de]),
                    )
                    nc.scalar.activation(
                        h1s[:, sl, dout : dout + 1],
                        ps[:, :, dout : dout + 1],
                        AF.Exp,
                        bias=bf1col[:, 0:1],
                        scale=k1,
                    )
                    # h2s = rho ⊙ h1s (col dout -> rho*F1 = F2) on gpsimd
                    nc.gpsimd.tensor_mul(
                        h2s[:, sl, :],
                        h1s[:, sl, :],
                        rhocols[:, sl, None].broadcast_to([128, grp, de]),
                    )

                # --- main loop: accumulate over the resident adjT tiles ---
                acc1 = psacc.tile([de, rows], F32)
                acc2 = psacc.tile([de, rows], F32)
                def acc_mm(acc, stat, mv, c0, c1, start, stop):
                    # matmul outputs may not cross the 512-col PSUM bank edge
                    for b0 in range(0, rows, 512):
                        lo = max(c0, b0)
                        hi = min(c1, b0 + 512)
                        if lo < hi:
                            nc.tensor.matmul(
                                acc[:, lo:hi],
                                stat,
                                mv[:, lo:hi],
                                start=start,
                                stop=stop,
                                skip_group_check=True,
                            )

                acc_mm(acc1, zstat, zmov, 0, rows, True, last_g1 is None)
                acc_mm(acc2, zstat, zmov, 0, rows, True, last_g2 is None)
                for m in range(nm):
                    adjt_t = adjt_tiles[m]
                    for t in range(mt):
                        g = mt * m + t
                        z2, z1 = Z2[g], Z1[g]
                        mv = adjt_t[:, t, :]
                        if z2 > 0:
                            acc_mm(
                                acc2, h2s[:, g, :], mv, 0, z2,
                                False, g == last_g2,
                            )
                        if z1 > z2:
                            # mixed columns: q_b = adj * sel(1, R_i*rho_j)
                            tb = bp.tile([128, z1 - z2], BF16, name="tb")
                            nc.vector.tensor_scalar(
                                tb,
                                Rbc[:, z2:z1],
                                rhocols[:, g : g + 1],
                                1.0,
                                OP.mult,
                                selop,
                            )
                            qb = bp.tile([128, z1 - z2], BF16, name="qb")
                            nc.vector.tensor_mul(qb, tb, mv[:, z2:z1])
                            for b0 in range(0, rows, 512):
                                lo = max(z2, b0)
                                hi = min(z1, b0 + 512)
                                if lo < hi:
                                    nc.tensor.matmul(
                                        acc1[:, lo:hi],
                                        h1s[:, g, :],
                                        qb[:, lo - z2 : hi - z2],
                                        start=False,
                                        stop=(
                                            g == last_g1 and z1 == rows
                                            and hi == z1
                                        ),
                                        skip_group_check=True,
                                    )
                        if z1 < rows:
                            acc_mm(
                                acc1, h1s[:, g, :], mv, z1, rows,
                                False, g == last_g1,
                            )

                # --- combine: hp = acc1 + R ⊙ acc2 (row dout = s) ---
                # split so the tail chunks can start before the full width
                # is combined
                t2 = consts.tile([de, rows], F32, name="combine_t2")
                for c0 in range(0, rows, 256):
                    cs = slice(c0, c0 + 256)
                    nc.vector.tensor_mul(t2[:, cs], acc2[:, cs], Rbc[0:de, cs])
                    nc.vector.tensor_add(hpT[:, cs], t2[:, cs], acc1[:, cs])

            with tc.tile_pool(name="pstail", bufs=4, space="PSUM") as pstail:
                for cc in range(rows // 128):
                    tp = pstail.tile([128, de], F32)
                    nc.tensor.transpose(
                        tp,
                        hpT[:, cc * 128 : (cc + 1) * 128],
                        identity[0:de, 0:de],
                    )
                    rcol = tailp.tile([128, 1], F32)
                    nc.vector.reciprocal(rcol, tp[:, dout:de])
                    # elu(v) = max(v, exp(min(v, 0)) - 1), v = hp * (1/s)
                    vm = tailp.tile([128, dout], F32)
                    nc.vector.tensor_scalar(
                        vm, tp[:, 0:dout], rcol[:, 0:1], 0.0,
                        OP.mult, OP.min,
                    )
                    e2 = tailp.tile([128, dout], F32)
                    nc.scalar.activation(e2, vm, AF.Exp, bias=zcol[:, 0:1])
                    ysb = tailp.tile([128, dout], F32)
                    nc.vector._custom_dve(
                        emx, out=ysb, in0=tp[:, 0:dout], in1=e2,
                        s0=rcol[:, 0:1], s1=0.0, imm2=0.0,
                    )
                    nc.sync.dma_start(
                        out=y_d[cc * 128 : (cc + 1) * 128, :], in_=ysb
                    )
    nc.compile()
    return nc


def _run(x, adj, w, a, a_coeff, b_coeff, c_coeff, d_coeff):
    global LAST_RESULTS, LAST_NC
    n, din = x.shape
    dout = w.shape[1]
    assert adj.shape == (n, n) and a.shape == (2 * dout, 1)
    rows = n // NCORES

    A = float(np.asarray(a_coeff).reshape(-1)[0])
    B = float(np.asarray(b_coeff).reshape(-1)[0])
    C = float(np.asarray(c_coeff).reshape(-1)[0])
    D0 = float(np.asarray(d_coeff).reshape(-1)[0])
    L1 = _leaky(A + B)
    assert C > 0.0, "kernel assumes c_coeff > 0"

    x = np.ascontiguousarray(x, dtype=np.float32)
    adj = np.asarray(adj, dtype=np.float32)
    assert ((adj == 0.0) | (adj == 1.0)).all(), "adj must be binary"
    w = np.ascontiguousarray(w, dtype=np.float32)
    a = np.ascontiguousarray(a, dtype=np.float32)

    h = x @ w
    hz1 = (h @ a[:dout, 0]).astype(np.float64)
    hz2 = (h @ a[dout:, 0]).astype(np.float64)

    k1 = L1 * C
    k2 = L1 * C * SLOPE
    d1 = L1 * D0
    d2 = L1 * SLOPE * D0
    s0 = k1 * float(hz2.max()) + d1 if k1 >= 0 else k1 * float(hz2.min()) + d1

    jperm = np.argsort(hz2, kind="stable")
    iperm = np.argsort(hz1, kind="stable")
    hz2_s = hz2[jperm]
    hz1_s = hz1[iperm]

    xt16 = np.ascontiguousarray(x.T).astype(np.float16)
    xt_j = np.ascontiguousarray(xt16[:, jperm])

    # per-group split points per core: y = C*(hz1_i + hz2_j) + D0 >= 0 is
    # branch-1; group g spans hz2_s[g*128 : (g+1)*128]
    ng = n // 128
    thr = -D0 / C
    consts_d = dict(
        k1=k1, k2=k2, b_f1=d1 - s0, b_rho=d2 - d1, sel_max=(L1 >= 0.0)
    )

    # Strided round-robin i-assignment: core c owns hz1-ranks c, c+8, ...
    # so every core's sorted-i column space samples hz1 near-identically and
    # one set of per-group column splits works for all cores.
    in_maps = []
    z_per_core = []
    for c in range(NCORES):
        iown = iperm[c::NCORES]
        h1own = hz1_s[c::NCORES]
        z2 = []
        z1 = []
        for g in range(ng):
            mn = hz2_s[g * 128]
            mx = hz2_s[(g + 1) * 128 - 1]
            # pure branch-2 (y<0 for all j in g): hz1_i < thr - mx
            # pure branch-1 (y>=0 for all j in g): hz1_i >= thr - mn
            lo = int(np.searchsorted(h1own, thr - mx, side="left"))
            hi = int(np.searchsorted(h1own, thr - mn, side="right"))
            z2.append(lo)
            z1.append(hi)
        z_per_core.append((z2, z1))
        adjt_enc = np.ascontiguousarray(
            adj[np.ix_(iown, jperm)].T
        ).astype(mybir.dt.np(F8E4))
        in_maps.append(
            {
                "adjt": adjt_enc,
                "xt": xt_j,
                "xt_own": np.ascontiguousarray(xt16[:, iown]),
                "w": w,
                "a": a,
            }
        )

    # all cores share one program: use the union splits (mixed region must
    # cover every core's mixed region; pure regions shrink to the common part)
    z2_u = [min(z_per_core[c][0][g] for c in range(NCORES)) for g in range(ng)]
    z1_u = [max(z_per_core[c][1][g] for c in range(NCORES)) for g in range(ng)]
    consts_d["z2"] = z2_u
    consts_d["z1"] = z1_u

    nc = _build(n, din, dout, rows, consts_d)
    LAST_NC = nc

    res = run_bass_kernel_spmd(
        nc, in_maps, core_ids=list(range(NCORES)), trace=TRACE
    )
    LAST_RESULTS = res
    y = np.empty((n, dout), dtype=np.float32)
    for c in range(NCORES):
        y[iperm[c::NCORES]] = res.results[c]["y"]
    return y


def kernel(x, adj, w, a, a_coeff, b_coeff, c_coeff, d_coeff):
    return _run(x, adj, w, a, a_coeff, b_coeff, c_coeff, d_coeff)
